# revision 17
# baseline (speedup 1.0000x reference)
"""Tensor-parallel TinyLlama prefill decoder on 8 Trainium2 NeuronCores.

Returns the stacked pre-RoPE KV cache [2, L, B, H, S, HD] (the only live
output of the reference's prefill forward; the final layer's attention/FFN
are dead code and are skipped).

Sharding: tensor-parallel over heads (2/core) and FFN columns (704/core);
norms replicated. The residual stream is chunked into 4 groups of 512
positions and the whole layer is software-pipelined over chunks: each
chunk's attention-out / FFN-down partial is AllReduced (fp16, 2 MB) as
soon as it is produced, while the tensor engine works on other chunks.
This hides the collective latency and keeps the PE HAM clock-gate warm.

Activations live transposed ([E, S]) in SBUF so every matmul contracts
along partitions without transposes; scores are computed transposed
([k, q]) so the softmax denominator falls out of the o-matmul via an
appended ones column on v.
"""

import os
from contextlib import ExitStack

import numpy as np

import concourse.bass as bass
import concourse.mybir as mybir
import concourse.tile as tile
from concourse import bacc
from concourse.bass_utils import run_bass_kernel_spmd

F16 = mybir.dt.float16
F32 = mybir.dt.float32
AF = mybir.ActivationFunctionType

# model config (hardcoded per contract)
B, S, E, H, HD, FF, L, V = 1, 2048, 2048, 16, 128, 5632, 4, 32000
ROPE_THETA = 10000.0
EPS = 1e-5
NC = 8                      # cores
HPC = H // NC               # heads per core (2)
DPC = HPC * HD              # qkv dims per core (256)
FPC = FF // NC              # ffn dims per core (704)
ET = E // 128               # E tiles (16)
ST = S // 128               # S blocks (16)
SC = 512                    # position-chunk width
NCH = S // SC               # chunks (4)
FT = 5                      # full 128-row FF tiles; plus one 64-row tile
SCALE = float(HD) ** -0.5

_CACHE = {}


def build_kernel():
    nc = bacc.Bacc("TRN2", target_bir_lowering=False, debug=False,
                   num_devices=NC)

    # ---- DRAM I/O --------------------------------------------------------
    x0T = nc.dram_tensor("x0T", [E, S], F16, kind="ExternalInput").ap()
    wq = nc.dram_tensor("wq", [L, E, DPC], F16, kind="ExternalInput").ap()
    wk = nc.dram_tensor("wk", [L, E, DPC], F16, kind="ExternalInput").ap()
    wv = nc.dram_tensor("wv", [L, E, DPC], F16, kind="ExternalInput").ap()
    wo = nc.dram_tensor("wo", [L, DPC, E], F16, kind="ExternalInput").ap()
    wg = nc.dram_tensor("wg", [L, E, FPC], F16, kind="ExternalInput").ap()
    wu = nc.dram_tensor("wu", [L, E, FPC], F16, kind="ExternalInput").ap()
    wd = nc.dram_tensor("wd", [L, FPC, E], F16, kind="ExternalInput").ap()
    cosT = nc.dram_tensor("cosT", [HD, S], F16, kind="ExternalInput").ap()
    sinT = nc.dram_tensor("sinT", [HD, S], F16, kind="ExternalInput").ap()
    rotP = nc.dram_tensor("rotP", [HD, HD], F16, kind="ExternalInput").ap()
    triM = nc.dram_tensor("triM", [128, 128], F16, kind="ExternalInput").ap()
    idnt = nc.dram_tensor("idnt", [128, 128], F16, kind="ExternalInput").ap()
    kvo = nc.dram_tensor("kv_out", [2, L, HPC, S, HD], F16,
                         kind="ExternalOutput").ap()

    with tile.TileContext(nc) as tc, ExitStack() as ctx:
        ctx.enter_context(nc.allow_low_precision(
            reason="fp16 kernel by design; accumulation stays fp32 in PSUM"))

        # ---- persistent SBUF ---------------------------------------------
        px = ctx.enter_context(tc.tile_pool(name="px", bufs=ET))
        x_t = []
        for e in range(ET):
            t = px.tile([128, S], F16, name=f"x_{e}", tag="x")
            nc.sync.dma_start(t[:], x0T[e * 128:(e + 1) * 128, :])
            x_t.append(t)

        pc = ctx.enter_context(tc.tile_pool(name="pconst", bufs=1))
        cos_sb = pc.tile([HD, S], F16, name="cos_sb")
        sin_sb = pc.tile([HD, S], F16, name="sin_sb")
        rot_sb = pc.tile([HD, HD], F16, name="rot_sb")
        tri_sb = pc.tile([128, 128], F16, name="tri_sb")
        id_sb = pc.tile([128, 128], F16, name="id_sb")
        ones_sb = pc.tile([128, 128], F16, name="ones_sb")
        eps_sb = pc.tile([128, 1], F32, name="eps_sb")
        nc.sync.dma_start(cos_sb[:], cosT[:])
        nc.sync.dma_start(sin_sb[:], sinT[:])
        nc.sync.dma_start(rot_sb[:], rotP[:])
        nc.sync.dma_start(tri_sb[:], triM[:])
        nc.sync.dma_start(id_sb[:], idnt[:])
        nc.gpsimd.memset(ones_sb[:], 1.0)
        nc.gpsimd.memset(eps_sb[:], EPS)

        # DRAM bounce buffers for the chunked AllReduces
        pdram = ctx.enter_context(tc.tile_pool(name="pdram", bufs=1,
                                               space="DRAM"))
        ar_in = [[pdram.tile([E, SC], F16, name=f"ar_in{ph}_{c}",
                             tag=f"ari{ph}{c}")
                  for c in range(NCH)] for ph in range(2)]
        # Shared DRAM outputs are single-writer: one tile per collective
        ar_out = {}
        for l in range(L - 1):
            for ph in range(2):
                for c in range(NCH):
                    ar_out[(l, ph, c)] = pdram.tile(
                        [E, SC], F16, name=f"ar_out{l}_{ph}_{c}",
                        addr_space="Shared", tag=f"aro{l}{ph}{c}")

        # ---- rotating work pools (SBUF) ----------------------------------
        pw = ctx.enter_context(tc.tile_pool(name="pw", bufs=2))
        pn = ctx.enter_context(tc.tile_pool(name="pn", bufs=2))
        pqk = ctx.enter_context(tc.tile_pool(name="pqk", bufs=1))
        pv = ctx.enter_context(tc.tile_pool(name="pv", bufs=17))
        pat = ctx.enter_context(tc.tile_pool(name="pat", bufs=3))
        pff = ctx.enter_context(tc.tile_pool(name="pff", bufs=2))
        pio = ctx.enter_context(tc.tile_pool(name="pio", bufs=2))

        # ---- PSUM pools (long-lived; ring-buffered by tag) ---------------
        # slots are bank-padded: 4 shared [128,512] f32 accumulators + 4
        # isolated o-accumulator banks = all 8 banks
        ppb = ctx.enter_context(tc.tile_pool(name="ppb", bufs=4,
                                             space="PSUM"))
        ppo = ctx.enter_context(tc.tile_pool(name="ppo", bufs=1,
                                             space="PSUM"))

        def mm512(nm):
            return ppb.tile([128, SC], F32, name=nm, tag="mm512", bufs=4)

        def rms_chunk(l, c, tag, want_rt):
            """R [128, SC] (rows all equal rsqrt(mean(x^2)+eps)) for
            position chunk c; optionally rT [128, 4] (per-partition r
            for each 128-block of the chunk)."""
            cs = slice(c * SC, (c + 1) * SC)
            ss = mm512(f"ss_{l}_{tag}_{c}")
            for e in range(ET):
                x2 = pn.tile([128, SC], F16, name=f"x2_{e}", tag="x2",
                             bufs=3)
                nc.vector.tensor_mul(x2[:], x_t[e][:, cs], x_t[e][:, cs])
                nc.tensor.matmul(ss[:], ones_sb[:], x2[:],
                                 start=(e == 0), stop=(e == ET - 1))
            sq = pn.tile([128, SC], F16, name=f"sq_{c}", tag="sq", bufs=2)
            nc.scalar.activation(sq[:], ss[:], AF.Sqrt,
                                 bias=eps_sb[:], scale=1.0 / E)
            R = pn.tile([128, SC], F16, name=f"R_{l}_{tag}_{c}",
                        tag=f"R{tag}", bufs=2)
            nc.vector.reciprocal(R[:], sq[:])
            rT = None
            if want_rt:
                # per-partition r for each 128-block, via DMA transpose
                rTh = pn.tile([128, 4 * 128], F16, name=f"rTh_{l}_{c}",
                              tag="rTh", bufs=2)
                rT = pn.tile([128, 4], F32, name=f"rT_{l}_{c}",
                             tag="rT", bufs=2)
                for j in range(4):
                    nc.sync.dma_start_transpose(
                        rTh[:, j * 128:(j + 1) * 128],
                        R[:, j * 128:(j + 1) * 128])
                    nc.vector.tensor_copy(rT[:, j:j + 1],
                                          rTh[:, j * 128:j * 128 + 1])
            return R, rT

        def load_w_cols(dram_ap, cols, name, tag, bufs):
            """DRAM [E, cols] -> SBUF [128, ET*cols], E-tile major."""
            t = pw.tile([128, ET * cols], F16, name=name, tag=tag, bufs=bufs)
            nc.sync.dma_start(
                t[:].rearrange("p (t m) -> p t m", t=ET),
                dram_ap.rearrange("(t p) m -> p t m", p=128))
            return t

        rg = [list(range(NC))]

        # persistent v tiles [s, d | ones]: ones cols written once, the
        # value region is overwritten every layer (WAR tracked by tile)
        vext = [[pv.tile([128, 132], F16, name=f"vx_{h}_{sb}",
                         tag=f"vx{h}", bufs=ST)
                 for sb in range(ST)] for h in range(HPC)]
        for h in range(HPC):
            for sb in range(ST):
                nc.gpsimd.memset(vext[h][sb][:, 128:132], 1.0)

        for l in range(L):
            act = l < L - 1

            # per-layer weights (ring of 4 slots: wk, wq, wv, wo)
            wk_sb = load_w_cols(wk[l], DPC, f"wk_sb_{l}", "wsm", 4)
            wq_sb = load_w_cols(wq[l], DPC, f"wq_sb_{l}", "wsm", 4) if act \
                else None
            wv_sb = load_w_cols(wv[l], DPC, f"wv_sb_{l}", "wsm", 4)
            wo_sb = None
            if act:
                wo_sb = pw.tile([128, HPC * E], F16, name=f"wo_sb_{l}",
                                tag="wsm", bufs=4)
                nc.sync.dma_start(
                    wo_sb[:].rearrange("p (t m) -> p t m", t=HPC),
                    wo[l].rearrange("(t p) m -> p t m", p=128))

            # per-layer k (rope'd) tiles, whole-S, written chunk by chunk
            kr_sb = [pqk.tile([128, S], F16, name=f"kr_{l}_{h}",
                              tag=f"kr{h}", bufs=1) for h in range(HPC)] \
                if act else [None] * HPC

            # ---- attention half: per chunk norm/qkv/attn/Wo + AR fire ----
            for c in range(NCH):
                cs = slice(c * SC, (c + 1) * SC)

                # residual add from previous layer's FFN AllReduce
                if l > 0:
                    for e in range(ET):
                        ld = pio.tile([128, SC], F16, name=f"arf_{e}",
                                      tag="arl", bufs=3)
                        nc.sync.dma_start(
                            ld[:],
                            ar_out[(l - 1, 1, c)][e * 128:(e + 1) * 128, :])
                        nc.vector.tensor_add(x_t[e][:, cs], x_t[e][:, cs],
                                             ld[:])

                R1, rT1 = rms_chunk(l, c, "a", want_rt=True)

                # q/k projections + RoPE + k output for this chunk
                srcs = [("k", wk_sb)] + ([("q", wq_sb)] if act else [])
                for nmw, wsb in srcs:
                    for h in range(HPC):
                        if act:
                            tgt = kr_sb[h] if nmw == "k" else None
                            if nmw == "q":
                                tgt = pqk.tile([128, SC], F16,
                                               name=f"qr_{l}_{h}_{c}",
                                               tag=f"qr{h}", bufs=2)
                        ps = mm512(f"qk_{nmw}_{h}_{c}")
                        for e in range(ET):
                            nc.tensor.matmul(
                                ps[:],
                                wsb[:, e * DPC + h * 128:
                                    e * DPC + (h + 1) * 128],
                                x_t[e][:, cs],
                                start=(e == 0), stop=(e == ET - 1))
                        raw = pn.tile([128, SC], F16, name=f"raw_{h}",
                                      tag="qkraw", bufs=3)
                        nc.vector.tensor_mul(raw[:], ps[:], R1[:])
                        if nmw == "k":
                            # k output (pre-RoPE): [d, s] -> [s, d]
                            for j in range(4):
                                sb = c * 4 + j
                                ko = pio.tile([128, 128], F16,
                                              name=f"kos_{sb}",
                                              tag="kosb", bufs=3)
                                nc.sync.dma_start_transpose(
                                    ko[:], raw[:, j * 128:(j + 1) * 128])
                                nc.sync.dma_start(
                                    kvo[0, l, h,
                                        sb * 128:(sb + 1) * 128, :],
                                    ko[:])
                        if act:
                            # RoPE: t = raw*cos + (rotP.T @ raw)*sin
                            dst = kr_sb[h][:, cs] if nmw == "k" else tgt[:]
                            rp = mm512(f"rot_{nmw}_{h}_{c}")
                            nc.tensor.matmul(rp[:], rot_sb[:], raw[:],
                                             start=True, stop=True)
                            nc.vector.tensor_mul(dst, raw[:], cos_sb[:, cs])
                            tmp = pn.tile([128, SC], F16, name=f"rtmp_{h}",
                                          tag="rtmp", bufs=2)
                            nc.vector.tensor_mul(tmp[:], rp[:],
                                                 sin_sb[:, cs])
                            nc.vector.tensor_add(dst, dst, tmp[:])
                            if nmw == "q":
                                qr_c = tgt
                                if h == 0:
                                    qr_sb = [None] * HPC
                                qr_sb[h] = qr_c

                # v for this chunk's 4 blocks, [s, d] + ones col + output
                for j in range(4):
                    sb = c * 4 + j
                    ps = mm512(f"v_{sb}")
                    for e in range(ET):
                        nc.tensor.matmul(
                            ps[:, 0:DPC], x_t[e][:, sb * 128:(sb + 1) * 128],
                            wv_sb[:, e * DPC:(e + 1) * DPC],
                            start=(e == 0), stop=(e == ET - 1))
                    for h in range(HPC):
                        vt = vext[h][sb]
                        nc.vector.tensor_scalar_mul(
                            vt[:, 0:128], ps[:, h * 128:(h + 1) * 128],
                            rT1[:, j:j + 1])
                        nc.sync.dma_start(
                            kvo[1, l, h, sb * 128:(sb + 1) * 128, :],
                            vt[:, 0:128])

                if not act:
                    continue

                # ---- attention for q-chunk c (scores transposed) --------
                oT = []
                for h in range(HPC):
                    ot = pqk.tile([128, SC], F16, name=f"oT_{l}_{h}_{c}",
                                  tag=f"oT{h}", bufs=2)
                    ops = [ppo.tile([128, 132], F32, name=f"ops{j}",
                                    tag=f"ops{j}", bufs=1)
                           for j in range(4)]
                    nkb = 4 * c + 4
                    for kb in range(nkb):
                        st = mm512(f"st_{h}_{kb}")
                        nc.tensor.matmul(
                            st[:], kr_sb[h][:, kb * 128:(kb + 1) * 128],
                            qr_sb[h][:], start=True, stop=True)
                        ex = pat.tile([128, SC], F16, name=f"ex_{kb}",
                                      tag="ex")
                        nc.scalar.activation(ex[:], st[:], AF.Exp,
                                             scale=SCALE)
                        for j in range(4):
                            qb = 4 * c + j
                            if qb < kb:
                                continue
                            exs = ex[:, j * 128:(j + 1) * 128]
                            if qb == kb:
                                nc.vector.tensor_mul(exs, exs, tri_sb[:])
                            nc.tensor.matmul(
                                ops[j], exs, vext[h][kb][:],
                                start=(kb == 0), stop=(kb == nkb - 1))
                    for j in range(4):
                        rec = pn.tile([128, 1], F32, name=f"rec{j}",
                                      tag="rec", bufs=2)
                        nc.vector.reciprocal(rec[:], ops[j][:, 128:129])
                        ob = pio.tile([128, 128], F16, name=f"ob{j}",
                                      tag="ob", bufs=3)
                        nc.vector.tensor_scalar_mul(ob[:], ops[j][:, 0:128],
                                                    rec[:])
                        nc.sync.dma_start_transpose(
                            ot[:, j * 128:(j + 1) * 128], ob[:])
                    oT.append(ot)

                # ---- Wo partial for chunk c + fire AllReduce ------------
                for m in range(ET):
                    ps = mm512(f"wo_{m}")
                    for h in range(HPC):
                        nc.tensor.matmul(
                            ps[:],
                            wo_sb[:, h * E + m * 128:h * E + (m + 1) * 128],
                            oT[h][:], start=(h == 0), stop=(h == HPC - 1))
                    cst = pio.tile([128, SC], F16, name=f"woc_{m}",
                                   tag="cast", bufs=3)
                    nc.scalar.copy(cst[:], ps[:])
                    nc.sync.dma_start(
                        ar_in[0][c][m * 128:(m + 1) * 128, :], cst[:])
                nc.gpsimd.collective_compute(
                    "AllReduce", mybir.AluOpType.add, replica_groups=rg,
                    ins=[ar_in[0][c].opt()], outs=[ar_out[(l, 0, c)].opt()])

            if not act:
                continue

            # ---- FFN half: per chunk residual/norm/gate-up/down + AR ----
            for c in range(NCH):
                cs = slice(c * SC, (c + 1) * SC)
                for e in range(ET):
                    ld = pio.tile([128, SC], F16, name=f"ara_{e}",
                                  tag="arl", bufs=3)
                    nc.sync.dma_start(
                        ld[:], ar_out[(l, 0, c)][e * 128:(e + 1) * 128, :])
                    nc.vector.tensor_add(x_t[e][:, cs], x_t[e][:, cs],
                                         ld[:])
                R2, _ = rms_chunk(l, c, "f", want_rt=False)

                m_sb = []
                for fm in range(FT + 1):
                    rows = 128 if fm < FT else FPC - FT * 128
                    wgs = pff.tile([128, ET * rows], F16, name=f"wgs{fm}",
                                   tag="wgs", bufs=2)
                    nc.sync.dma_start(
                        wgs[:].rearrange("p (t m) -> p t m", t=ET),
                        wg[l][:, fm * 128:fm * 128 + rows].rearrange(
                            "(t p) m -> p t m", p=128))
                    wus = pff.tile([128, ET * rows], F16, name=f"wus{fm}",
                                   tag="wus", bufs=2)
                    nc.sync.dma_start(
                        wus[:].rearrange("p (t m) -> p t m", t=ET),
                        wu[l][:, fm * 128:fm * 128 + rows].rearrange(
                            "(t p) m -> p t m", p=128))
                    gp = mm512(f"g_{fm}_{c}")
                    up = mm512(f"u_{fm}_{c}")
                    for e in range(ET):
                        nc.tensor.matmul(
                            gp[0:rows, :], wgs[:, e * rows:(e + 1) * rows],
                            x_t[e][:, cs], start=(e == 0),
                            stop=(e == ET - 1))
                    for e in range(ET):
                        nc.tensor.matmul(
                            up[0:rows, :], wus[:, e * rows:(e + 1) * rows],
                            x_t[e][:, cs], start=(e == 0),
                            stop=(e == ET - 1))
                    gs = pff.tile([128, SC], F16, name=f"gs{fm}", tag="gs",
                                  bufs=2)
                    us = pff.tile([128, SC], F16, name=f"us{fm}", tag="us",
                                  bufs=2)
                    mt = pff.tile([128, SC], F16, name=f"m_{fm}_{c}",
                                  tag="mff", bufs=12)
                    nc.vector.tensor_mul(gs[0:rows, :], gp[0:rows, :],
                                         R2[0:rows, :])
                    nc.scalar.activation(gs[0:rows, :], gs[0:rows, :],
                                         AF.Silu)
                    nc.vector.tensor_mul(us[0:rows, :], up[0:rows, :],
                                         R2[0:rows, :])
                    nc.vector.tensor_mul(mt[0:rows, :], gs[0:rows, :],
                                         us[0:rows, :])
                    m_sb.append(mt)

                # down-proj partial for chunk c
                for m in range(ET):
                    wds = pff.tile([128, (FT + 1) * 128], F16,
                                   name=f"wds{m}", tag="wds", bufs=3)
                    nc.sync.dma_start(
                        wds[:, 0:FT * 128].rearrange("p (t m) -> p t m",
                                                     t=FT),
                        wd[l][0:FT * 128, m * 128:(m + 1) * 128].rearrange(
                            "(t p) m -> p t m", p=128))
                    nc.sync.dma_start(
                        wds[0:FPC - FT * 128, FT * 128:(FT + 1) * 128],
                        wd[l][FT * 128:FPC, m * 128:(m + 1) * 128])
                    ps = mm512(f"dn_{m}")
                    for fm in range(FT + 1):
                        rows = 128 if fm < FT else FPC - FT * 128
                        nc.tensor.matmul(
                            ps[:], wds[0:rows, fm * 128:(fm + 1) * 128],
                            m_sb[fm][0:rows, :],
                            start=(fm == 0), stop=(fm == FT))
                    cst = pio.tile([128, SC], F16, name=f"dnc_{m}",
                                   tag="cast", bufs=3)
                    nc.vector.tensor_copy(cst[:], ps[:])
                    nc.sync.dma_start(
                        ar_in[1][c][m * 128:(m + 1) * 128, :], cst[:])
                nc.gpsimd.collective_compute(
                    "AllReduce", mybir.AluOpType.add, replica_groups=rg,
                    ins=[ar_in[1][c].opt()], outs=[ar_out[(l, 1, c)].opt()])

    nc.compile()
    return nc


def _host_prep(inputs):
    """Fold norms into weights, build tables, TP-shard -> per-core in_maps."""
    ids = np.asarray(inputs["input_ids"]).reshape(-1)
    x0 = np.asarray(inputs["embed"])[ids]          # [S, E] fp32
    x0T = np.ascontiguousarray(x0.T).astype(np.float16)

    ln1 = np.asarray(inputs["ln1"], dtype=np.float32)   # [L, E]
    ln2 = np.asarray(inputs["ln2"], dtype=np.float32)
    wq_f = ln1[:, :, None] * np.asarray(inputs["Wq"])   # [L, E, H*HD]
    wk_f = ln1[:, :, None] * np.asarray(inputs["Wk"])
    wv_f = ln1[:, :, None] * np.asarray(inputs["Wv"])
    wg_f = ln2[:, :, None] * np.asarray(inputs["Wg"])
    wu_f = ln2[:, :, None] * np.asarray(inputs["Wu"])
    wo_f = np.asarray(inputs["Wo"])                     # [L, H*HD, E]
    wd_f = np.asarray(inputs["Wd"])                     # [L, FF, E]

    inv = 1.0 / (ROPE_THETA ** (np.arange(0, HD, 2, dtype=np.float32) / HD))
    t = np.arange(S, dtype=np.float32)
    freqs = np.outer(t, inv)                       # [S, HD/2]
    emb = np.concatenate([freqs, freqs], axis=-1)  # [S, HD]
    cosT = np.ascontiguousarray(np.cos(emb).T).astype(np.float16)
    sinT = np.ascontiguousarray(np.sin(emb).T).astype(np.float16)

    rotP = np.zeros((HD, HD), dtype=np.float16)
    half = HD // 2
    for d in range(half):
        rotP[d + half, d] = -1.0
    for d in range(half, HD):
        rotP[d - half, d] = 1.0

    triM = np.triu(np.ones((128, 128), dtype=np.float16))   # [k, q] valid
    idnt = np.eye(128, dtype=np.float16)

    in_maps = []
    for c in range(NC):
        ds = slice(c * DPC, (c + 1) * DPC)
        fs = slice(c * FPC, (c + 1) * FPC)
        in_maps.append({
            "x0T": x0T,
            "wq": np.ascontiguousarray(wq_f[:, :, ds]).astype(np.float16),
            "wk": np.ascontiguousarray(wk_f[:, :, ds]).astype(np.float16),
            "wv": np.ascontiguousarray(wv_f[:, :, ds]).astype(np.float16),
            "wo": np.ascontiguousarray(wo_f[:, ds, :]).astype(np.float16),
            "wg": np.ascontiguousarray(wg_f[:, :, fs]).astype(np.float16),
            "wu": np.ascontiguousarray(wu_f[:, :, fs]).astype(np.float16),
            "wd": np.ascontiguousarray(wd_f[:, fs, :]).astype(np.float16),
            "cosT": cosT, "sinT": sinT, "rotP": rotP,
            "triM": triM, "idnt": idnt,
        })
    return in_maps


def kernel(**inputs):
    if "nc" not in _CACHE:
        _CACHE["nc"] = build_kernel()
    nc = _CACHE["nc"]
    in_maps = _host_prep(inputs)
    trace = os.environ.get("KERNEL_TRACE") == "1"
    res = run_bass_kernel_spmd(nc, in_maps, core_ids=list(range(NC)),
                               trace=trace)
    if trace and res.exec_time_ns is not None:
        print(f"HW exec time: {res.exec_time_ns} ns")
        _CACHE["exec_time_ns"] = res.exec_time_ns
        if res.instructions_and_trace:
            print("trace:", res.instructions_and_trace[1])

    out = np.zeros((2, L, B, H, S, HD), dtype=np.float32)
    for c in range(NC):
        kv = res.results[c]["kv_out"].astype(np.float32)  # [2, L, HPC, S, HD]
        for h in range(HPC):
            out[:, :, 0, c * HPC + h] = kv[:, :, h]
    return out


# revision 21
# speedup vs baseline: 1.2494x; 1.2494x over previous
"""Tensor-parallel TinyLlama prefill decoder on 8 Trainium2 NeuronCores.

Returns the stacked pre-RoPE KV cache [2, L, B, H, S, HD] (the only live
output of the reference's prefill forward; the final layer's attention/FFN
are dead code and are skipped).

Sharding: tensor-parallel over heads (2/core) and FFN columns (704/core);
norms replicated. The residual stream is chunked into 4 groups of 512
positions and the whole layer is software-pipelined over chunks: each
chunk's attention-out / FFN-down partial is AllReduced (fp16, 2 MB) as
soon as it is produced, while the tensor engine works on other chunks.
This hides the collective latency and keeps the PE HAM clock-gate warm.

Activations live transposed ([E, S]) in SBUF so every matmul contracts
along partitions without transposes; scores are computed transposed
([k, q]) so the softmax denominator falls out of the o-matmul via an
appended ones column on v.
"""

import os
from contextlib import ExitStack

import numpy as np

import concourse.bass as bass
import concourse.mybir as mybir
import concourse.tile as tile
from concourse import bacc
from concourse.bass_utils import run_bass_kernel_spmd

F16 = mybir.dt.float16
F32 = mybir.dt.float32
AF = mybir.ActivationFunctionType

# model config (hardcoded per contract)
B, S, E, H, HD, FF, L, V = 1, 2048, 2048, 16, 128, 5632, 4, 32000
ROPE_THETA = 10000.0
EPS = 1e-5
NC = 8                      # cores
HPC = H // NC               # heads per core (2)
DPC = HPC * HD              # qkv dims per core (256)
FPC = FF // NC              # ffn dims per core (704)
ET = E // 128               # E tiles (16)
ST = S // 128               # S blocks (16)
SC = 512                    # position-chunk width
NCH = S // SC               # chunks (4)
FT = 5                      # full 128-row FF tiles; plus one 64-row tile
SCALE = float(HD) ** -0.5

_CACHE = {}


def build_kernel():
    nc = bacc.Bacc("TRN2", target_bir_lowering=False, debug=False,
                   num_devices=NC)

    # ---- DRAM I/O --------------------------------------------------------
    x0T = nc.dram_tensor("x0T", [E, S], F16, kind="ExternalInput").ap()
    wq = nc.dram_tensor("wq", [L, E, DPC], F16, kind="ExternalInput").ap()
    wk = nc.dram_tensor("wk", [L, E, DPC], F16, kind="ExternalInput").ap()
    wv = nc.dram_tensor("wv", [L, E, DPC], F16, kind="ExternalInput").ap()
    wo = nc.dram_tensor("wo", [L, DPC, E], F16, kind="ExternalInput").ap()
    wg = nc.dram_tensor("wg", [L, E, FPC], F16, kind="ExternalInput").ap()
    wu = nc.dram_tensor("wu", [L, E, FPC], F16, kind="ExternalInput").ap()
    wd = nc.dram_tensor("wd", [L, FPC, E], F16, kind="ExternalInput").ap()
    cosT = nc.dram_tensor("cosT", [HD, S], F16, kind="ExternalInput").ap()
    sinT = nc.dram_tensor("sinT", [HD, S], F16, kind="ExternalInput").ap()
    rotP = nc.dram_tensor("rotP", [HD, HD], F16, kind="ExternalInput").ap()
    triM = nc.dram_tensor("triM", [128, 128], F16, kind="ExternalInput").ap()
    idnt = nc.dram_tensor("idnt", [128, 128], F16, kind="ExternalInput").ap()
    kvo = nc.dram_tensor("kv_out", [2, L, HPC, S, HD], F16,
                         kind="ExternalOutput").ap()

    with tile.TileContext(nc) as tc, ExitStack() as ctx:
        ctx.enter_context(nc.allow_low_precision(
            reason="fp16 kernel by design; accumulation stays fp32 in PSUM"))

        # ---- persistent SBUF ---------------------------------------------
        px = ctx.enter_context(tc.tile_pool(name="px", bufs=ET))
        x_t = []
        for e in range(ET):
            t = px.tile([128, S], F16, name=f"x_{e}", tag="x")
            nc.sync.dma_start(t[:], x0T[e * 128:(e + 1) * 128, :])
            x_t.append(t)

        pc = ctx.enter_context(tc.tile_pool(name="pconst", bufs=1))
        cos_sb = pc.tile([HD, S], F16, name="cos_sb")
        sin_sb = pc.tile([HD, S], F16, name="sin_sb")
        rot_sb = pc.tile([HD, HD], F16, name="rot_sb")
        tri_sb = pc.tile([128, 128], F16, name="tri_sb")
        id_sb = pc.tile([128, 128], F16, name="id_sb")
        ones_sb = pc.tile([128, 128], F16, name="ones_sb")
        eps_sb = pc.tile([128, 1], F32, name="eps_sb")
        nc.sync.dma_start(cos_sb[:], cosT[:])
        nc.sync.dma_start(sin_sb[:], sinT[:])
        nc.sync.dma_start(rot_sb[:], rotP[:])
        nc.sync.dma_start(tri_sb[:], triM[:])
        nc.sync.dma_start(id_sb[:], idnt[:])
        nc.gpsimd.memset(ones_sb[:], 1.0)
        nc.gpsimd.memset(eps_sb[:], EPS)

        # DRAM bounce buffers for the chunked AllReduces
        pdram = ctx.enter_context(tc.tile_pool(name="pdram", bufs=1,
                                               space="DRAM"))
        ar_in = [[pdram.tile([E, SC], F16, name=f"ar_in{ph}_{c}",
                             tag=f"ari{ph}{c}")
                  for c in range(NCH)] for ph in range(2)]
        # Shared DRAM outputs are single-writer: one tile per collective
        ar_out = {}
        for l in range(L - 1):
            for ph in range(2):
                for c in range(NCH):
                    ar_out[(l, ph, c)] = pdram.tile(
                        [E, SC], F16, name=f"ar_out{l}_{ph}_{c}",
                        addr_space="Shared", tag=f"aro{l}{ph}{c}")

        # ---- rotating work pools (SBUF) ----------------------------------
        pw = ctx.enter_context(tc.tile_pool(name="pw", bufs=2))
        pn = ctx.enter_context(tc.tile_pool(name="pn", bufs=2))
        pqk = ctx.enter_context(tc.tile_pool(name="pqk", bufs=1))
        pv = ctx.enter_context(tc.tile_pool(name="pv", bufs=17))
        pat = ctx.enter_context(tc.tile_pool(name="pat", bufs=3))
        pff = ctx.enter_context(tc.tile_pool(name="pff", bufs=2))
        pio = ctx.enter_context(tc.tile_pool(name="pio", bufs=2))

        # ---- PSUM pools (long-lived; ring-buffered by tag) ---------------
        # slots are bank-padded: 3 shared [128,512] f32 accumulators + 4
        # isolated o-accumulator banks + 1 transpose bank = all 8 banks
        ppb = ctx.enter_context(tc.tile_pool(name="ppb", bufs=3,
                                             space="PSUM"))
        ppo = ctx.enter_context(tc.tile_pool(name="ppo", bufs=1,
                                             space="PSUM"))
        pps = ctx.enter_context(tc.tile_pool(name="pps", bufs=1,
                                             space="PSUM"))

        def mm512(nm):
            return ppb.tile([128, SC], F32, name=nm, tag="mm512", bufs=3)

        def tp128(nm):
            return pps.tile([128, 128], F16, name=nm, tag="tp", bufs=1)

        def rms_chunk(l, c, tag, want_rt):
            """R [128, SC] (rows all equal rsqrt(mean(x^2)+eps)) for
            position chunk c; optionally rT [128, 4] (per-partition r
            for each 128-block of the chunk)."""
            cs = slice(c * SC, (c + 1) * SC)
            ss = mm512(f"ss_{l}_{tag}_{c}")
            for e in range(ET):
                x2 = pn.tile([128, SC], F16, name=f"x2_{e}", tag="x2",
                             bufs=3)
                nc.vector.tensor_mul(x2[:], x_t[e][:, cs], x_t[e][:, cs])
                nc.tensor.matmul(ss[:], ones_sb[:], x2[:],
                                 start=(e == 0), stop=(e == ET - 1))
            sq = pn.tile([128, SC], F16, name=f"sq_{c}", tag="sq", bufs=2)
            nc.scalar.activation(sq[:], ss[:], AF.Sqrt,
                                 bias=eps_sb[:], scale=1.0 / E)
            R = pn.tile([128, SC], F16, name=f"R_{l}_{tag}_{c}",
                        tag=f"R{tag}", bufs=2)
            nc.vector.reciprocal(R[:], sq[:])
            rT = None
            if want_rt:
                # per-partition r for each 128-block, via PE transpose
                rT = pn.tile([128, 4], F32, name=f"rT_{l}_{c}",
                             tag="rT", bufs=2)
                for j in range(4):
                    tp = tp128(f"tpr{j}")
                    nc.tensor.transpose(
                        tp[:], R[:, j * 128:(j + 1) * 128], id_sb[:])
                    nc.vector.tensor_copy(rT[:, j:j + 1], tp[:, 0:1])
            return R, rT

        def load_w_cols(dram_ap, cols, name, tag, bufs):
            """DRAM [E, cols] -> SBUF [128, ET*cols], E-tile major."""
            t = pw.tile([128, ET * cols], F16, name=name, tag=tag, bufs=bufs)
            nc.sync.dma_start(
                t[:].rearrange("p (t m) -> p t m", t=ET),
                dram_ap.rearrange("(t p) m -> p t m", p=128))
            return t

        rg = [list(range(NC))]

        # persistent v tiles [s, d | ones]: ones cols written once, the
        # value region is overwritten every layer (WAR tracked by tile)
        vext = [[pv.tile([128, 132], F16, name=f"vx_{h}_{sb}",
                         tag=f"vx{h}", bufs=ST)
                 for sb in range(ST)] for h in range(HPC)]
        for h in range(HPC):
            for sb in range(ST):
                nc.gpsimd.memset(vext[h][sb][:, 128:132], 1.0)

        for l in range(L):
            act = l < L - 1

            # per-layer weights (ring of 4 slots: wk, wq, wv, wo)
            wk_sb = load_w_cols(wk[l], DPC, f"wk_sb_{l}", "wsm", 4)
            wq_sb = load_w_cols(wq[l], DPC, f"wq_sb_{l}", "wsm", 4) if act \
                else None
            wv_sb = load_w_cols(wv[l], DPC, f"wv_sb_{l}", "wsm", 4)
            wo_sb = None
            if act:
                wo_sb = pw.tile([128, HPC * E], F16, name=f"wo_sb_{l}",
                                tag="wsm", bufs=4)
                nc.sync.dma_start(
                    wo_sb[:].rearrange("p (t m) -> p t m", t=HPC),
                    wo[l].rearrange("(t p) m -> p t m", p=128))

            # per-layer k (rope'd) tiles, whole-S, written chunk by chunk
            kr_sb = [pqk.tile([128, S], F16, name=f"kr_{l}_{h}",
                              tag=f"kr{h}", bufs=1) for h in range(HPC)] \
                if act else [None] * HPC

            # ---- attention half: per chunk norm/qkv/attn/Wo + AR fire ----
            for c in range(NCH):
                cs = slice(c * SC, (c + 1) * SC)

                # residual add from previous layer's FFN AllReduce
                if l > 0:
                    for e in range(ET):
                        ld = pio.tile([128, SC], F16, name=f"arf_{e}",
                                      tag="arl", bufs=3)
                        nc.sync.dma_start(
                            ld[:],
                            ar_out[(l - 1, 1, c)][e * 128:(e + 1) * 128, :])
                        nc.vector.tensor_add(x_t[e][:, cs], x_t[e][:, cs],
                                             ld[:])

                R1, rT1 = rms_chunk(l, c, "a", want_rt=True)

                # q/k projections + RoPE + k output for this chunk
                srcs = [("k", wk_sb)] + ([("q", wq_sb)] if act else [])
                for nmw, wsb in srcs:
                    for h in range(HPC):
                        if act:
                            tgt = kr_sb[h] if nmw == "k" else None
                            if nmw == "q":
                                tgt = pqk.tile([128, SC], F16,
                                               name=f"qr_{l}_{h}_{c}",
                                               tag=f"qr{h}", bufs=2)
                        ps = mm512(f"qk_{nmw}_{h}_{c}")
                        for e in range(ET):
                            nc.tensor.matmul(
                                ps[:],
                                wsb[:, e * DPC + h * 128:
                                    e * DPC + (h + 1) * 128],
                                x_t[e][:, cs],
                                start=(e == 0), stop=(e == ET - 1))
                        raw = pn.tile([128, SC], F16, name=f"raw_{h}",
                                      tag="qkraw", bufs=3)
                        nc.vector.tensor_mul(raw[:], ps[:], R1[:])
                        if nmw == "k":
                            # k output (pre-RoPE): [d, s] -> [s, d]
                            for j in range(4):
                                sb = c * 4 + j
                                tp = tp128(f"ko{j}")
                                nc.tensor.transpose(
                                    tp[:], raw[:, j * 128:(j + 1) * 128],
                                    id_sb[:])
                                ko = pio.tile([128, 128], F16,
                                              name=f"kos_{sb}",
                                              tag="kosb", bufs=3)
                                nc.vector.tensor_copy(ko[:], tp[:])
                                nc.sync.dma_start(
                                    kvo[0, l, h,
                                        sb * 128:(sb + 1) * 128, :],
                                    ko[:])
                        if act:
                            # RoPE: t = raw*cos + (rotP.T @ raw)*sin
                            dst = kr_sb[h][:, cs] if nmw == "k" else tgt[:]
                            rp = mm512(f"rot_{nmw}_{h}_{c}")
                            nc.tensor.matmul(rp[:], rot_sb[:], raw[:],
                                             start=True, stop=True)
                            nc.vector.tensor_mul(dst, raw[:], cos_sb[:, cs])
                            tmp = pn.tile([128, SC], F16, name=f"rtmp_{h}",
                                          tag="rtmp", bufs=2)
                            nc.vector.tensor_mul(tmp[:], rp[:],
                                                 sin_sb[:, cs])
                            nc.vector.tensor_add(dst, dst, tmp[:])
                            if nmw == "q":
                                qr_c = tgt
                                if h == 0:
                                    qr_sb = [None] * HPC
                                qr_sb[h] = qr_c

                # v for this chunk's 4 blocks, [s, d] + ones col + output
                for j in range(4):
                    sb = c * 4 + j
                    ps = mm512(f"v_{sb}")
                    for e in range(ET):
                        nc.tensor.matmul(
                            ps[:, 0:DPC], x_t[e][:, sb * 128:(sb + 1) * 128],
                            wv_sb[:, e * DPC:(e + 1) * DPC],
                            start=(e == 0), stop=(e == ET - 1))
                    for h in range(HPC):
                        vt = vext[h][sb]
                        nc.vector.tensor_scalar_mul(
                            vt[:, 0:128], ps[:, h * 128:(h + 1) * 128],
                            rT1[:, j:j + 1])
                        nc.sync.dma_start(
                            kvo[1, l, h, sb * 128:(sb + 1) * 128, :],
                            vt[:, 0:128])

                if not act:
                    continue

                # ---- attention for q-chunk c (scores transposed) --------
                oT = []
                for h in range(HPC):
                    ot = pqk.tile([128, SC], F16, name=f"oT_{l}_{h}_{c}",
                                  tag=f"oT{h}", bufs=2)
                    ops = [ppo.tile([128, 132], F32, name=f"ops{j}",
                                    tag=f"ops{j}", bufs=1)
                           for j in range(4)]
                    nkb = 4 * c + 4
                    for kb in range(nkb):
                        st = mm512(f"st_{h}_{kb}")
                        nc.tensor.matmul(
                            st[:], kr_sb[h][:, kb * 128:(kb + 1) * 128],
                            qr_sb[h][:], start=True, stop=True)
                        ex = pat.tile([128, SC], F16, name=f"ex_{kb}",
                                      tag="ex")
                        nc.scalar.activation(ex[:], st[:], AF.Exp,
                                             scale=SCALE)
                        for j in range(4):
                            qb = 4 * c + j
                            if qb < kb:
                                continue
                            exs = ex[:, j * 128:(j + 1) * 128]
                            if qb == kb:
                                nc.vector.tensor_mul(exs, exs, tri_sb[:])
                            nc.tensor.matmul(
                                ops[j], exs, vext[h][kb][:],
                                start=(kb == 0), stop=(kb == nkb - 1))
                    for j in range(4):
                        rec = pn.tile([128, 1], F32, name=f"rec{j}",
                                      tag="rec", bufs=2)
                        nc.vector.reciprocal(rec[:], ops[j][:, 128:129])
                        ob = pio.tile([128, 128], F16, name=f"ob{j}",
                                      tag="ob", bufs=3)
                        nc.vector.tensor_scalar_mul(ob[:], ops[j][:, 0:128],
                                                    rec[:])
                        tp = tp128(f"to{j}")
                        nc.tensor.transpose(tp[:], ob[:], id_sb[:])
                        nc.vector.tensor_copy(
                            ot[:, j * 128:(j + 1) * 128], tp[:])
                    oT.append(ot)

                # ---- Wo partial for chunk c + fire AllReduce ------------
                for m in range(ET):
                    ps = mm512(f"wo_{m}")
                    for h in range(HPC):
                        nc.tensor.matmul(
                            ps[:],
                            wo_sb[:, h * E + m * 128:h * E + (m + 1) * 128],
                            oT[h][:], start=(h == 0), stop=(h == HPC - 1))
                    cst = pio.tile([128, SC], F16, name=f"woc_{m}",
                                   tag="cast", bufs=3)
                    nc.scalar.copy(cst[:], ps[:])
                    nc.sync.dma_start(
                        ar_in[0][c][m * 128:(m + 1) * 128, :], cst[:])
                nc.gpsimd.collective_compute(
                    "AllReduce", mybir.AluOpType.add, replica_groups=rg,
                    ins=[ar_in[0][c].opt()], outs=[ar_out[(l, 0, c)].opt()])

            if not act:
                continue

            # ---- FFN half: per chunk residual/norm/gate-up/down + AR ----
            for c in range(NCH):
                cs = slice(c * SC, (c + 1) * SC)
                for e in range(ET):
                    ld = pio.tile([128, SC], F16, name=f"ara_{e}",
                                  tag="arl", bufs=3)
                    nc.sync.dma_start(
                        ld[:], ar_out[(l, 0, c)][e * 128:(e + 1) * 128, :])
                    nc.vector.tensor_add(x_t[e][:, cs], x_t[e][:, cs],
                                         ld[:])
                R2, _ = rms_chunk(l, c, "f", want_rt=False)

                m_sb = []
                for fm in range(FT + 1):
                    rows = 128 if fm < FT else FPC - FT * 128
                    wgs = pff.tile([128, ET * rows], F16, name=f"wgs{fm}",
                                   tag="wgs", bufs=2)
                    nc.sync.dma_start(
                        wgs[:].rearrange("p (t m) -> p t m", t=ET),
                        wg[l][:, fm * 128:fm * 128 + rows].rearrange(
                            "(t p) m -> p t m", p=128))
                    wus = pff.tile([128, ET * rows], F16, name=f"wus{fm}",
                                   tag="wus", bufs=2)
                    nc.sync.dma_start(
                        wus[:].rearrange("p (t m) -> p t m", t=ET),
                        wu[l][:, fm * 128:fm * 128 + rows].rearrange(
                            "(t p) m -> p t m", p=128))
                    gp = mm512(f"g_{fm}_{c}")
                    up = mm512(f"u_{fm}_{c}")
                    for e in range(ET):
                        nc.tensor.matmul(
                            gp[0:rows, :], wgs[:, e * rows:(e + 1) * rows],
                            x_t[e][:, cs], start=(e == 0),
                            stop=(e == ET - 1))
                    for e in range(ET):
                        nc.tensor.matmul(
                            up[0:rows, :], wus[:, e * rows:(e + 1) * rows],
                            x_t[e][:, cs], start=(e == 0),
                            stop=(e == ET - 1))
                    gs = pff.tile([128, SC], F16, name=f"gs{fm}", tag="gs",
                                  bufs=2)
                    us = pff.tile([128, SC], F16, name=f"us{fm}", tag="us",
                                  bufs=2)
                    mt = pff.tile([128, SC], F16, name=f"m_{fm}_{c}",
                                  tag="mff", bufs=12)
                    nc.vector.tensor_mul(gs[0:rows, :], gp[0:rows, :],
                                         R2[0:rows, :])
                    nc.scalar.activation(gs[0:rows, :], gs[0:rows, :],
                                         AF.Silu)
                    nc.vector.tensor_mul(us[0:rows, :], up[0:rows, :],
                                         R2[0:rows, :])
                    nc.vector.tensor_mul(mt[0:rows, :], gs[0:rows, :],
                                         us[0:rows, :])
                    m_sb.append(mt)

                # down-proj partial for chunk c
                for m in range(ET):
                    wds = pff.tile([128, (FT + 1) * 128], F16,
                                   name=f"wds{m}", tag="wds", bufs=3)
                    nc.sync.dma_start(
                        wds[:, 0:FT * 128].rearrange("p (t m) -> p t m",
                                                     t=FT),
                        wd[l][0:FT * 128, m * 128:(m + 1) * 128].rearrange(
                            "(t p) m -> p t m", p=128))
                    nc.sync.dma_start(
                        wds[0:FPC - FT * 128, FT * 128:(FT + 1) * 128],
                        wd[l][FT * 128:FPC, m * 128:(m + 1) * 128])
                    ps = mm512(f"dn_{m}")
                    for fm in range(FT + 1):
                        rows = 128 if fm < FT else FPC - FT * 128
                        nc.tensor.matmul(
                            ps[:], wds[0:rows, fm * 128:(fm + 1) * 128],
                            m_sb[fm][0:rows, :],
                            start=(fm == 0), stop=(fm == FT))
                    cst = pio.tile([128, SC], F16, name=f"dnc_{m}",
                                   tag="cast", bufs=3)
                    nc.vector.tensor_copy(cst[:], ps[:])
                    nc.sync.dma_start(
                        ar_in[1][c][m * 128:(m + 1) * 128, :], cst[:])
                nc.gpsimd.collective_compute(
                    "AllReduce", mybir.AluOpType.add, replica_groups=rg,
                    ins=[ar_in[1][c].opt()], outs=[ar_out[(l, 1, c)].opt()])

    nc.compile()
    return nc


def _host_prep(inputs):
    """Fold norms into weights, build tables, TP-shard -> per-core in_maps."""
    ids = np.asarray(inputs["input_ids"]).reshape(-1)
    x0 = np.asarray(inputs["embed"])[ids]          # [S, E] fp32
    x0T = np.ascontiguousarray(x0.T).astype(np.float16)

    ln1 = np.asarray(inputs["ln1"], dtype=np.float32)   # [L, E]
    ln2 = np.asarray(inputs["ln2"], dtype=np.float32)
    wq_f = ln1[:, :, None] * np.asarray(inputs["Wq"])   # [L, E, H*HD]
    wk_f = ln1[:, :, None] * np.asarray(inputs["Wk"])
    wv_f = ln1[:, :, None] * np.asarray(inputs["Wv"])
    wg_f = ln2[:, :, None] * np.asarray(inputs["Wg"])
    wu_f = ln2[:, :, None] * np.asarray(inputs["Wu"])
    wo_f = np.asarray(inputs["Wo"])                     # [L, H*HD, E]
    wd_f = np.asarray(inputs["Wd"])                     # [L, FF, E]

    inv = 1.0 / (ROPE_THETA ** (np.arange(0, HD, 2, dtype=np.float32) / HD))
    t = np.arange(S, dtype=np.float32)
    freqs = np.outer(t, inv)                       # [S, HD/2]
    emb = np.concatenate([freqs, freqs], axis=-1)  # [S, HD]
    cosT = np.ascontiguousarray(np.cos(emb).T).astype(np.float16)
    sinT = np.ascontiguousarray(np.sin(emb).T).astype(np.float16)

    rotP = np.zeros((HD, HD), dtype=np.float16)
    half = HD // 2
    for d in range(half):
        rotP[d + half, d] = -1.0
    for d in range(half, HD):
        rotP[d - half, d] = 1.0

    triM = np.triu(np.ones((128, 128), dtype=np.float16))   # [k, q] valid
    idnt = np.eye(128, dtype=np.float16)

    in_maps = []
    for c in range(NC):
        ds = slice(c * DPC, (c + 1) * DPC)
        fs = slice(c * FPC, (c + 1) * FPC)
        in_maps.append({
            "x0T": x0T,
            "wq": np.ascontiguousarray(wq_f[:, :, ds]).astype(np.float16),
            "wk": np.ascontiguousarray(wk_f[:, :, ds]).astype(np.float16),
            "wv": np.ascontiguousarray(wv_f[:, :, ds]).astype(np.float16),
            "wo": np.ascontiguousarray(wo_f[:, ds, :]).astype(np.float16),
            "wg": np.ascontiguousarray(wg_f[:, :, fs]).astype(np.float16),
            "wu": np.ascontiguousarray(wu_f[:, :, fs]).astype(np.float16),
            "wd": np.ascontiguousarray(wd_f[:, fs, :]).astype(np.float16),
            "cosT": cosT, "sinT": sinT, "rotP": rotP,
            "triM": triM, "idnt": idnt,
        })
    return in_maps


def kernel(**inputs):
    if "nc" not in _CACHE:
        _CACHE["nc"] = build_kernel()
    nc = _CACHE["nc"]
    in_maps = _host_prep(inputs)
    trace = os.environ.get("KERNEL_TRACE") == "1"
    res = run_bass_kernel_spmd(nc, in_maps, core_ids=list(range(NC)),
                               trace=trace)
    if trace and res.exec_time_ns is not None:
        print(f"HW exec time: {res.exec_time_ns} ns")
        _CACHE["exec_time_ns"] = res.exec_time_ns
        if res.instructions_and_trace:
            print("trace:", res.instructions_and_trace[1])

    out = np.zeros((2, L, B, H, S, HD), dtype=np.float32)
    for c in range(NC):
        kv = res.results[c]["kv_out"].astype(np.float32)  # [2, L, HPC, S, HD]
        for h in range(HPC):
            out[:, :, 0, c * HPC + h] = kv[:, :, h]
    return out


# revision 45
# speedup vs baseline: 1.3545x; 1.0841x over previous
"""Tensor-parallel TinyLlama prefill decoder on 8 Trainium2 NeuronCores.

Returns the stacked pre-RoPE KV cache [2, L, B, H, S, HD] (the only live
output of the reference's prefill forward; the final layer's attention/FFN
are dead code and are skipped).

Sharding: tensor-parallel over heads (2/core) and FFN columns (704/core);
norms replicated. The residual stream is chunked into 4 groups of 512
positions and the whole layer is software-pipelined over chunks: each
chunk's attention-out / FFN-down partial is AllReduced (fp16, 2 MB) as
soon as it is produced, while the tensor engine works on other chunks.
This hides the collective latency and keeps the PE HAM clock-gate warm.

Activations live transposed ([E, S]) in SBUF so every matmul contracts
along partitions without transposes; scores are computed transposed
([k, q]) so the softmax denominator falls out of the o-matmul via an
appended ones column on v.
"""

import os
from contextlib import ExitStack

import numpy as np

import concourse.bass as bass
import concourse.mybir as mybir
import concourse.tile as tile
from concourse import bacc
from concourse.bass_utils import run_bass_kernel_spmd

F16 = mybir.dt.float16
F32 = mybir.dt.float32
F8 = mybir.dt.float8e4
DR = mybir.MatmulPerfMode.DoubleRow
AF = mybir.ActivationFunctionType

# model config (hardcoded per contract)
B, S, E, H, HD, FF, L, V = 1, 2048, 2048, 16, 128, 5632, 4, 32000
ROPE_THETA = 10000.0
EPS = 1e-5
NC = 8                      # cores
HPC = H // NC               # heads per core (2)
DPC = HPC * HD              # qkv dims per core (256)
FPC = FF // NC              # ffn dims per core (704)
ET = E // 128               # E tiles (16)
ST = S // 128               # S blocks (16)
SC = 512                    # position-chunk width
NCH = S // SC               # chunks (4)
FT = 5                      # full 128-row FF tiles; plus one 64-row tile
FPCP = 768                  # FPC zero-padded to 6 full tiles (fp8 pairing)
FT6 = FPCP // 128           # 6
WS = 128.0                  # fp8 weight scale (w*WS in fp8, descale after)
SCALE = float(HD) ** -0.5

_CACHE = {}


def build_kernel():
    nc = bacc.Bacc("TRN2", target_bir_lowering=False, debug=False,
                   num_devices=NC)

    # ---- DRAM I/O --------------------------------------------------------
    x0T = nc.dram_tensor("x0T", [E, S], F16, kind="ExternalInput").ap()
    wq = nc.dram_tensor("wq", [L, E, DPC], F16, kind="ExternalInput").ap()
    wk = nc.dram_tensor("wk", [L, E, DPC], F16, kind="ExternalInput").ap()
    wv = nc.dram_tensor("wv", [L, E, DPC], F16, kind="ExternalInput").ap()
    wo = nc.dram_tensor("wo", [L, DPC, E], F16, kind="ExternalInput").ap()
    wg = nc.dram_tensor("wg", [L, E, FPC], F16, kind="ExternalInput").ap()
    wu = nc.dram_tensor("wu", [L, E, FPC], F16, kind="ExternalInput").ap()
    wd = nc.dram_tensor("wd", [L, FPC, E], F16, kind="ExternalInput").ap()
    cosT = nc.dram_tensor("cosT", [HD, S], F16, kind="ExternalInput").ap()
    sinT = nc.dram_tensor("sinT", [HD, S], F16, kind="ExternalInput").ap()
    rotP = nc.dram_tensor("rotP", [HD, HD], F16, kind="ExternalInput").ap()
    triM = nc.dram_tensor("triM", [128, 128], F16, kind="ExternalInput").ap()
    idnt = nc.dram_tensor("idnt", [128, 128], F16, kind="ExternalInput").ap()
    kvo = nc.dram_tensor("kv_out", [2, L, HPC, S, HD], F16,
                         kind="ExternalOutput").ap()

    with tile.TileContext(nc) as tc, ExitStack() as ctx:
        ctx.enter_context(nc.allow_low_precision(
            reason="fp16 kernel by design; accumulation stays fp32 in PSUM"))

        # ---- persistent SBUF ---------------------------------------------
        px = ctx.enter_context(tc.tile_pool(name="px", bufs=ET))
        x_t = []
        for e in range(ET):
            t = px.tile([128, S], F16, name=f"x_{e}", tag="x")
            nc.sync.dma_start(t[:], x0T[e * 128:(e + 1) * 128, :])
            x_t.append(t)

        pc = ctx.enter_context(tc.tile_pool(name="pconst", bufs=1))
        cos_sb = pc.tile([HD, S], F16, name="cos_sb")
        sin_sb = pc.tile([HD, S], F16, name="sin_sb")
        rot_sb = pc.tile([HD, HD], F16, name="rot_sb")
        tri_sb = pc.tile([128, 128], F16, name="tri_sb")
        id_sb = pc.tile([128, 128], F16, name="id_sb")
        ones_sb = pc.tile([128, 128], F16, name="ones_sb")
        eps_sb = pc.tile([128, 1], F32, name="eps_sb")
        nc.sync.dma_start(cos_sb[:], cosT[:])
        nc.sync.dma_start(sin_sb[:], sinT[:])
        nc.sync.dma_start(rot_sb[:], rotP[:])
        nc.sync.dma_start(tri_sb[:], triM[:])
        nc.sync.dma_start(id_sb[:], idnt[:])
        nc.gpsimd.memset(ones_sb[:], 1.0)
        nc.gpsimd.memset(eps_sb[:], EPS)

        # DRAM bounce buffers for the chunked AllReduces
        pdram = ctx.enter_context(tc.tile_pool(name="pdram", bufs=1,
                                               space="DRAM"))
        ar_in = [[pdram.tile([E, SC], F16, name=f"ar_in{ph}_{c}",
                             tag=f"ari{ph}{c}")
                  for c in range(NCH)] for ph in range(2)]
        # Shared DRAM outputs are single-writer: one tile per collective
        ar_out = {}
        for l in range(L - 1):
            for ph in range(2):
                for c in range(NCH):
                    ar_out[(l, ph, c)] = pdram.tile(
                        [E, SC], F16, name=f"ar_out{l}_{ph}_{c}",
                        addr_space="Shared", tag=f"aro{l}{ph}{c}")

        # ---- rotating work pools (SBUF) ----------------------------------
        pw = ctx.enter_context(tc.tile_pool(name="pw", bufs=2))
        pn = ctx.enter_context(tc.tile_pool(name="pn", bufs=2))
        pqk = ctx.enter_context(tc.tile_pool(name="pqk", bufs=1))
        pv = ctx.enter_context(tc.tile_pool(name="pv", bufs=17))
        pat = ctx.enter_context(tc.tile_pool(name="pat", bufs=3))
        pff = ctx.enter_context(tc.tile_pool(name="pff", bufs=2))
        pio = ctx.enter_context(tc.tile_pool(name="pio", bufs=2))

        # ---- PSUM pools (long-lived; ring-buffered by tag) ---------------
        # slots are bank-padded: 4 shared [128,512] f32 accumulators + 4
        # o-accumulator banks (doubling as transpose scratch) = 8 banks
        ppb = ctx.enter_context(tc.tile_pool(name="ppb", bufs=4,
                                             space="PSUM"))
        ppo = ctx.enter_context(tc.tile_pool(name="ppo", bufs=1,
                                             space="PSUM"))

        def mm512(nm):
            return ppb.tile([128, SC], F32, name=nm, tag="mm512", bufs=4)

        def tp128(j, nm):
            return ppo.tile([128, 128], F16, name=nm, tag=f"ops{j}",
                            bufs=1)

        def rms_chunk(l, c, tag, want_rt):
            """R [128, SC] (rows all equal rsqrt(mean(x^2)+eps)) for
            position chunk c; optionally rT [128, 4] (per-partition r
            for each 128-block of the chunk)."""
            cs = slice(c * SC, (c + 1) * SC)
            ss = mm512(f"ss_{l}_{tag}_{c}")
            for e in range(ET):
                x2 = pn.tile([128, SC], F16, name=f"x2_{e}", tag="x2",
                             bufs=3)
                nc.vector.tensor_mul(x2[:], x_t[e][:, cs], x_t[e][:, cs])
                nc.tensor.matmul(ss[:], ones_sb[:], x2[:],
                                 start=(e == 0), stop=(e == ET - 1))
            R = pn.tile([128, SC], F16, name=f"R_{l}_{tag}_{c}",
                        tag=f"R{tag}", bufs=2)
            nc.scalar.activation(R[:], ss[:], AF.Abs_reciprocal_sqrt,
                                 bias=eps_sb[:], scale=1.0 / E)
            rT = None
            if want_rt:
                # per-partition r for each 128-block, via PE transpose
                rT = pn.tile([128, 4], F32, name=f"rT_{l}_{c}",
                             tag="rT", bufs=2)
                for j in range(4):
                    tp = tp128(j, f"tpr{j}")
                    nc.tensor.transpose(
                        tp[:], R[:, j * 128:(j + 1) * 128], id_sb[:])
                    nc.vector.tensor_copy(rT[:, j:j + 1], tp[:, 0:1])
            return R, rT

        def load_w_cols(dram_ap, cols, name, tag, bufs):
            """DRAM [E, cols] -> SBUF [128, ET*cols], E-tile major."""
            t = pw.tile([128, ET * cols], F16, name=name, tag=tag, bufs=bufs)
            nc.sync.dma_start(
                t[:].rearrange("p (t m) -> p t m", t=ET),
                dram_ap.rearrange("(t p) m -> p t m", p=128))
            return t

        rg = [list(range(NC))]

        # persistent v tiles [s, d | ones]: ones cols written once, the
        # value region is overwritten every layer (WAR tracked by tile)
        vext = [[pv.tile([128, 132], F16, name=f"vx_{h}_{sb}",
                         tag=f"vx{h}", bufs=ST)
                 for sb in range(ST)] for h in range(HPC)]
        for h in range(HPC):
            for sb in range(ST):
                nc.gpsimd.memset(vext[h][sb][:, 128:132], 1.0)

        for l in range(L):
            act = l < L - 1

            # per-layer weights (ring of 4 slots: wk, wq, wv, wo)
            wk_sb = load_w_cols(wk[l], DPC, f"wk_sb_{l}", "wsm", 4)
            wq_sb = load_w_cols(wq[l], DPC, f"wq_sb_{l}", "wsm", 4) if act \
                else None
            wv_sb = load_w_cols(wv[l], DPC, f"wv_sb_{l}", "wsm", 4)
            wo_sb = None
            if act:
                wo_sb = pw.tile([128, HPC * E], F16, name=f"wo_sb_{l}",
                                tag="wsm", bufs=4)
                nc.sync.dma_start(
                    wo_sb[:].rearrange("p (t m) -> p t m", t=HPC),
                    wo[l].rearrange("(t p) m -> p t m", p=128))

            # per-layer k (rope'd) tiles, whole-S, written chunk by chunk
            kr_sb = [pqk.tile([128, S], F16, name=f"kr_{l}_{h}",
                              tag=f"kr{h}", bufs=1) for h in range(HPC)] \
                if act else [None] * HPC

            # ---- attention half: per chunk norm/qkv/attn/Wo + AR fire ----
            for c in range(NCH):
                cs = slice(c * SC, (c + 1) * SC)

                # residual add from previous layer's FFN AllReduce
                if l > 0:
                    for g in range(4):
                        ld4 = pio.tile([128, 4, SC], F16, name=f"arf_{g}",
                                       tag="arl", bufs=2)
                        nc.sync.dma_start(
                            ld4[:],
                            ar_out[(l - 1, 1, c)][g * 512:(g + 1) * 512, :]
                            .rearrange("(t p) m -> p t m", p=128))
                        for i in range(4):
                            e = 4 * g + i
                            nc.vector.tensor_add(x_t[e][:, cs],
                                                 x_t[e][:, cs],
                                                 ld4[:, i, :])

                R1, rT1 = rms_chunk(l, c, "a", want_rt=True)

                # q/k projections + RoPE + k output for this chunk
                srcs = [("k", wk_sb)] + ([("q", wq_sb)] if act else [])
                for nmw, wsb in srcs:
                    for h in range(HPC):
                        if act:
                            tgt = kr_sb[h] if nmw == "k" else None
                            if nmw == "q":
                                tgt = pqk.tile([128, SC], F16,
                                               name=f"qr_{l}_{h}_{c}",
                                               tag=f"qr{h}", bufs=2)
                        ps = mm512(f"qk_{nmw}_{h}_{c}")
                        for e in range(ET):
                            nc.tensor.matmul(
                                ps[:],
                                wsb[:, e * DPC + h * 128:
                                    e * DPC + (h + 1) * 128],
                                x_t[e][:, cs],
                                start=(e == 0), stop=(e == ET - 1))
                        raw = pn.tile([128, SC], F16, name=f"raw_{h}",
                                      tag="qkraw", bufs=3)
                        nc.vector.tensor_mul(raw[:], ps[:], R1[:])
                        if nmw == "k":
                            # k output (pre-RoPE): [d, s] -> [s, d]
                            for j in range(4):
                                sb = c * 4 + j
                                tp = tp128(j, f"ko{j}")
                                nc.tensor.transpose(
                                    tp[:], raw[:, j * 128:(j + 1) * 128],
                                    id_sb[:])
                                ko = pio.tile([128, 128], F16,
                                              name=f"kos_{sb}",
                                              tag="kosb", bufs=3)
                                nc.vector.tensor_copy(ko[:], tp[:])
                                nc.sync.dma_start(
                                    kvo[0, l, h,
                                        sb * 128:(sb + 1) * 128, :],
                                    ko[:])
                        if act:
                            # RoPE: t = raw*cos + (rotP.T @ raw)*sin
                            dst = kr_sb[h][:, cs] if nmw == "k" else tgt[:]
                            rp = mm512(f"rot_{nmw}_{h}_{c}")
                            nc.tensor.matmul(rp[:], rot_sb[:], raw[:],
                                             start=True, stop=True)
                            nc.vector.tensor_mul(dst, raw[:], cos_sb[:, cs])
                            tmp = pn.tile([128, SC], F16, name=f"rtmp_{h}",
                                          tag="rtmp", bufs=2)
                            nc.vector.tensor_mul(tmp[:], rp[:],
                                                 sin_sb[:, cs])
                            nc.vector.tensor_add(dst, dst, tmp[:])
                            if nmw == "q":
                                qr_c = tgt
                                if h == 0:
                                    qr_sb = [None] * HPC
                                qr_sb[h] = qr_c

                # v for this chunk's 4 blocks, [s, d] + ones col + output
                for j in range(4):
                    sb = c * 4 + j
                    ps = mm512(f"v_{sb}")
                    for e in range(ET):
                        nc.tensor.matmul(
                            ps[:, 0:DPC], x_t[e][:, sb * 128:(sb + 1) * 128],
                            wv_sb[:, e * DPC:(e + 1) * DPC],
                            start=(e == 0), stop=(e == ET - 1))
                    for h in range(HPC):
                        vt = vext[h][sb]
                        nc.vector.tensor_scalar_mul(
                            vt[:, 0:128], ps[:, h * 128:(h + 1) * 128],
                            rT1[:, j:j + 1])
                        nc.sync.dma_start(
                            kvo[1, l, h, sb * 128:(sb + 1) * 128, :],
                            vt[:, 0:128])

                if not act:
                    continue

                # ---- attention for q-chunk c (scores transposed) --------
                oT = []
                for h in range(HPC):
                    ot = pqk.tile([128, SC], F16, name=f"oT_{l}_{h}_{c}",
                                  tag=f"oT{h}", bufs=2)
                    ops = [ppo.tile([128, 132], F32, name=f"ops{j}",
                                    tag=f"ops{j}", bufs=1)
                           for j in range(4)]
                    nkb = 4 * c + 4
                    for kb in range(nkb):
                        st = mm512(f"st_{h}_{kb}")
                        nc.tensor.matmul(
                            st[:], kr_sb[h][:, kb * 128:(kb + 1) * 128],
                            qr_sb[h][:], start=True, stop=True)
                        ex = pat.tile([128, SC], F16, name=f"ex_{kb}",
                                      tag="ex")
                        nc.scalar.activation(ex[:], st[:], AF.Exp,
                                             scale=SCALE)
                        for j in range(4):
                            qb = 4 * c + j
                            if qb < kb:
                                continue
                            exs = ex[:, j * 128:(j + 1) * 128]
                            if qb == kb:
                                nc.vector.tensor_mul(exs, exs, tri_sb[:])
                            nc.tensor.matmul(
                                ops[j], exs, vext[h][kb][:],
                                start=(kb == 0), stop=(kb == nkb - 1))
                    for j in range(4):
                        rec = pn.tile([128, 1], F32, name=f"rec{j}",
                                      tag="rec", bufs=2)
                        nc.vector.reciprocal(rec[:], ops[j][:, 128:129])
                        ob = pio.tile([128, 128], F16, name=f"ob{j}",
                                      tag="ob", bufs=3)
                        nc.vector.tensor_scalar_mul(ob[:], ops[j][:, 0:128],
                                                    rec[:])
                        tp = tp128(j, f"to{j}")
                        nc.tensor.transpose(tp[:], ob[:], id_sb[:])
                        nc.vector.tensor_copy(
                            ot[:, j * 128:(j + 1) * 128], tp[:])
                    oT.append(ot)

                # ---- Wo partial for chunk c + fire AllReduce ------------
                for m in range(ET):
                    ps = mm512(f"wo_{m}")
                    for h in range(HPC):
                        nc.tensor.matmul(
                            ps[:],
                            wo_sb[:, h * E + m * 128:h * E + (m + 1) * 128],
                            oT[h][:], start=(h == 0), stop=(h == HPC - 1))
                    cst = pio.tile([128, SC], F16, name=f"woc_{m}",
                                   tag="cast", bufs=3)
                    nc.scalar.copy(cst[:], ps[:])
                    nc.scalar.dma_start(
                        ar_in[0][c][m * 128:(m + 1) * 128, :], cst[:])
                nc.gpsimd.collective_compute(
                    "AllReduce", mybir.AluOpType.add, replica_groups=rg,
                    ins=[ar_in[0][c].opt()], outs=[ar_out[(l, 0, c)].opt()])

            if not act:
                continue

            # ---- FFN half: chunk-PAIRS share one weight load ------------
            for half in range(NCH // 2):
                cpair = (2 * half, 2 * half + 1)
                R2s = {}
                for c in cpair:
                    cs = slice(c * SC, (c + 1) * SC)
                    for g in range(4):
                        ld4 = pio.tile([128, 4, SC], F16, name=f"ara_{g}",
                                       tag="arl", bufs=2)
                        nc.sync.dma_start(
                            ld4[:],
                            ar_out[(l, 0, c)][g * 512:(g + 1) * 512, :]
                            .rearrange("(t p) m -> p t m", p=128))
                        for i in range(4):
                            e = 4 * g + i
                            nc.vector.tensor_add(x_t[e][:, cs],
                                                 x_t[e][:, cs],
                                                 ld4[:, i, :])
                    R2s[c], _ = rms_chunk(l, c, "f", want_rt=False)

                m_sb = {c: [] for c in cpair}
                for fm in range(FT + 1):
                    rows = 128 if fm < FT else FPC - FT * 128
                    wgs = pff.tile([128, ET * rows], F16, name=f"wgs{fm}",
                                   tag="wgs", bufs=2)
                    nc.sync.dma_start(
                        wgs[:].rearrange("p (t m) -> p t m", t=ET),
                        wg[l][:, fm * 128:fm * 128 + rows].rearrange(
                            "(t p) m -> p t m", p=128))
                    wus = pff.tile([128, ET * rows], F16, name=f"wus{fm}",
                                   tag="wus", bufs=2)
                    nc.sync.dma_start(
                        wus[:].rearrange("p (t m) -> p t m", t=ET),
                        wu[l][:, fm * 128:fm * 128 + rows].rearrange(
                            "(t p) m -> p t m", p=128))
                    pp = {}
                    for c in cpair:
                        cs = slice(c * SC, (c + 1) * SC)
                        gp = mm512(f"g_{fm}_{c}")
                        up = mm512(f"u_{fm}_{c}")
                        for e in range(ET):
                            nc.tensor.matmul(
                                gp[0:rows, :],
                                wgs[:, e * rows:(e + 1) * rows],
                                x_t[e][:, cs], start=(e == 0),
                                stop=(e == ET - 1))
                        for e in range(ET):
                            nc.tensor.matmul(
                                up[0:rows, :],
                                wus[:, e * rows:(e + 1) * rows],
                                x_t[e][:, cs], start=(e == 0),
                                stop=(e == ET - 1))
                        pp[c] = (gp, up)
                    for c in cpair:
                        gp, up = pp[c]
                        gs = pff.tile([128, SC], F16, name=f"gs{fm}",
                                      tag="gs", bufs=2)
                        us = pff.tile([128, SC], F16, name=f"us{fm}",
                                      tag="us", bufs=2)
                        mt = pff.tile([128, SC], F16, name=f"m_{fm}_{c}",
                                      tag="mff", bufs=13)
                        nc.vector.tensor_mul(gs[0:rows, :], gp[0:rows, :],
                                             R2s[c][0:rows, :])
                        nc.scalar.activation(gs[0:rows, :], gs[0:rows, :],
                                             AF.Silu)
                        nc.vector.tensor_mul(us[0:rows, :], up[0:rows, :],
                                             R2s[c][0:rows, :])
                        nc.vector.tensor_mul(mt[0:rows, :], gs[0:rows, :],
                                             us[0:rows, :])
                        m_sb[c].append(mt)

                # down-proj partials (wd loaded 4 m-cols at a time,
                # reused by both chunks of the pair)
                for g in range(4):
                    # [p, fm-tile, 4 m-blocks x 128] — 3D-balanceable DMA
                    wds4 = pff.tile([128, FT + 1, 512], F16,
                                    name=f"wds{g}", tag="wds", bufs=2)
                    nc.sync.dma_start(
                        wds4[:, 0:FT, :],
                        wd[l][0:FT * 128, g * 512:(g + 1) * 512]
                        .rearrange("(t p) m -> p t m", p=128))
                    nc.sync.dma_start(
                        wds4[0:FPC - FT * 128, FT, :],
                        wd[l][FT * 128:FPC, g * 512:(g + 1) * 512])
                    for c in cpair:
                        for i in range(4):
                            m = 4 * g + i
                            ps = mm512(f"dn_{m}")
                            for fm in range(FT + 1):
                                rows = 128 if fm < FT else FPC - FT * 128
                                nc.tensor.matmul(
                                    ps[:],
                                    wds4[0:rows, fm,
                                         i * 128:(i + 1) * 128],
                                    m_sb[c][fm][0:rows, :],
                                    start=(fm == 0), stop=(fm == FT))
                            cst = pio.tile([128, SC], F16, name=f"dnc_{m}",
                                           tag="cast", bufs=3)
                            nc.scalar.copy(cst[:], ps[:])
                            nc.scalar.dma_start(
                                ar_in[1][c][m * 128:(m + 1) * 128, :],
                                cst[:])
                for c in cpair:
                    nc.gpsimd.collective_compute(
                        "AllReduce", mybir.AluOpType.add, replica_groups=rg,
                        ins=[ar_in[1][c].opt()],
                        outs=[ar_out[(l, 1, c)].opt()])

    nc.compile()
    return nc


def _host_prep(inputs):
    """Fold norms into weights, build tables, TP-shard -> per-core in_maps."""
    ids = np.asarray(inputs["input_ids"]).reshape(-1)
    x0 = np.asarray(inputs["embed"])[ids]          # [S, E] fp32
    x0T = np.ascontiguousarray(x0.T).astype(np.float16)

    ln1 = np.asarray(inputs["ln1"], dtype=np.float32)   # [L, E]
    ln2 = np.asarray(inputs["ln2"], dtype=np.float32)
    wq_f = ln1[:, :, None] * np.asarray(inputs["Wq"])   # [L, E, H*HD]
    wk_f = ln1[:, :, None] * np.asarray(inputs["Wk"])
    wv_f = ln1[:, :, None] * np.asarray(inputs["Wv"])
    wg_f = ln2[:, :, None] * np.asarray(inputs["Wg"])
    wu_f = ln2[:, :, None] * np.asarray(inputs["Wu"])
    wo_f = np.asarray(inputs["Wo"])                     # [L, H*HD, E]
    wd_f = np.asarray(inputs["Wd"])                     # [L, FF, E]

    inv = 1.0 / (ROPE_THETA ** (np.arange(0, HD, 2, dtype=np.float32) / HD))
    t = np.arange(S, dtype=np.float32)
    freqs = np.outer(t, inv)                       # [S, HD/2]
    emb = np.concatenate([freqs, freqs], axis=-1)  # [S, HD]
    cosT = np.ascontiguousarray(np.cos(emb).T).astype(np.float16)
    sinT = np.ascontiguousarray(np.sin(emb).T).astype(np.float16)

    rotP = np.zeros((HD, HD), dtype=np.float16)
    half = HD // 2
    for d in range(half):
        rotP[d + half, d] = -1.0
    for d in range(half, HD):
        rotP[d - half, d] = 1.0

    triM = np.triu(np.ones((128, 128), dtype=np.float16))   # [k, q] valid
    idnt = np.eye(128, dtype=np.float16)

    in_maps = []
    for c in range(NC):
        ds = slice(c * DPC, (c + 1) * DPC)
        fs = slice(c * FPC, (c + 1) * FPC)
        in_maps.append({
            "x0T": x0T,
            "wq": np.ascontiguousarray(wq_f[:, :, ds]).astype(np.float16),
            "wk": np.ascontiguousarray(wk_f[:, :, ds]).astype(np.float16),
            "wv": np.ascontiguousarray(wv_f[:, :, ds]).astype(np.float16),
            "wo": np.ascontiguousarray(wo_f[:, ds, :]).astype(np.float16),
            "wg": np.ascontiguousarray(wg_f[:, :, fs]).astype(np.float16),
            "wu": np.ascontiguousarray(wu_f[:, :, fs]).astype(np.float16),
            "wd": np.ascontiguousarray(wd_f[:, fs, :]).astype(np.float16),
            "cosT": cosT, "sinT": sinT, "rotP": rotP,
            "triM": triM, "idnt": idnt,
        })
    return in_maps


def kernel(**inputs):
    if "nc" not in _CACHE:
        _CACHE["nc"] = build_kernel()
    nc = _CACHE["nc"]
    in_maps = _host_prep(inputs)
    trace = os.environ.get("KERNEL_TRACE") == "1"
    res = run_bass_kernel_spmd(nc, in_maps, core_ids=list(range(NC)),
                               trace=trace)
    if trace and res.exec_time_ns is not None:
        print(f"HW exec time: {res.exec_time_ns} ns")
        _CACHE["exec_time_ns"] = res.exec_time_ns
        if res.instructions_and_trace:
            print("trace:", res.instructions_and_trace[1])

    out = np.zeros((2, L, B, H, S, HD), dtype=np.float32)
    for c in range(NC):
        kv = res.results[c]["kv_out"].astype(np.float32)  # [2, L, HPC, S, HD]
        for h in range(HPC):
            out[:, :, 0, c * HPC + h] = kv[:, :, h]
    return out


# revision 51
# speedup vs baseline: 1.4546x; 1.0739x over previous
"""Tensor-parallel TinyLlama prefill decoder on 8 Trainium2 NeuronCores.

Returns the stacked pre-RoPE KV cache [2, L, B, H, S, HD] (the only live
output of the reference's prefill forward; the final layer's attention/FFN
are dead code and are skipped).

Sharding: tensor-parallel over heads (2/core) and FFN columns (704/core);
norms replicated. The residual stream is chunked into 4 groups of 512
positions and the whole layer is software-pipelined over chunks: each
chunk's attention-out / FFN-down partial is AllReduced (fp16, 2 MB) as
soon as it is produced, while the tensor engine works on other chunks.
This hides the collective latency and keeps the PE HAM clock-gate warm.

Activations live transposed ([E, S]) in SBUF so every matmul contracts
along partitions without transposes; scores are computed transposed
([k, q]) so the softmax denominator falls out of the o-matmul via an
appended ones column on v.
"""

import os
from contextlib import ExitStack

import numpy as np

import concourse.bass as bass
import concourse.mybir as mybir
import concourse.tile as tile
from concourse import bacc
from concourse.bass_utils import run_bass_kernel_spmd

F16 = mybir.dt.float16
F32 = mybir.dt.float32
F8 = mybir.dt.float8e4
DR = mybir.MatmulPerfMode.DoubleRow
AF = mybir.ActivationFunctionType

# model config (hardcoded per contract)
B, S, E, H, HD, FF, L, V = 1, 2048, 2048, 16, 128, 5632, 4, 32000
ROPE_THETA = 10000.0
EPS = 1e-5
NC = 8                      # cores
HPC = H // NC               # heads per core (2)
DPC = HPC * HD              # qkv dims per core (256)
FPC = FF // NC              # ffn dims per core (704)
ET = E // 128               # E tiles (16)
ST = S // 128               # S blocks (16)
SC = 512                    # position-chunk width
NCH = S // SC               # chunks (4)
FT = 5                      # full 128-row FF tiles; plus one 64-row tile
FPCP = 768                  # FPC zero-padded to 6 full tiles (fp8 pairing)
FT6 = FPCP // 128           # 6
WS = 128.0                  # fp8 weight scale (w*WS in fp8, descale after)
SCALE = float(HD) ** -0.5

_CACHE = {}


def build_kernel():
    nc = bacc.Bacc("TRN2", target_bir_lowering=False, debug=False,
                   num_devices=NC)

    # ---- DRAM I/O --------------------------------------------------------
    x0T = nc.dram_tensor("x0T", [E, S], F16, kind="ExternalInput").ap()
    wq = nc.dram_tensor("wq", [L, E, DPC], F16, kind="ExternalInput").ap()
    wk = nc.dram_tensor("wk", [L, E, DPC], F16, kind="ExternalInput").ap()
    wv = nc.dram_tensor("wv", [L, E, DPC], F16, kind="ExternalInput").ap()
    wo = nc.dram_tensor("wo", [L, DPC, E], F16, kind="ExternalInput").ap()
    wg = nc.dram_tensor("wg", [L, E, FPC], F16, kind="ExternalInput").ap()
    wu = nc.dram_tensor("wu", [L, E, FPC], F16, kind="ExternalInput").ap()
    wd = nc.dram_tensor("wd", [L, FPC, E], F16, kind="ExternalInput").ap()
    cosT = nc.dram_tensor("cosT", [HD, S], F16, kind="ExternalInput").ap()
    sinT = nc.dram_tensor("sinT", [HD, S], F16, kind="ExternalInput").ap()
    rotP = nc.dram_tensor("rotP", [HD, HD], F16, kind="ExternalInput").ap()
    triM = nc.dram_tensor("triM", [128, 128], F16, kind="ExternalInput").ap()
    idnt = nc.dram_tensor("idnt", [128, 128], F16, kind="ExternalInput").ap()
    kvo = nc.dram_tensor("kv_out", [2, L, HPC, S, HD], F16,
                         kind="ExternalOutput").ap()

    with tile.TileContext(nc) as tc, ExitStack() as ctx:
        ctx.enter_context(nc.allow_low_precision(
            reason="fp16 kernel by design; accumulation stays fp32 in PSUM"))

        # ---- persistent SBUF ---------------------------------------------
        px = ctx.enter_context(tc.tile_pool(name="px", bufs=ET))
        x_t = []
        for e in range(ET):
            t = px.tile([128, S], F16, name=f"x_{e}", tag="x")
            nc.sync.dma_start(t[:], x0T[e * 128:(e + 1) * 128, :])
            x_t.append(t)

        pc = ctx.enter_context(tc.tile_pool(name="pconst", bufs=1))
        cos_sb = pc.tile([HD, S], F16, name="cos_sb")
        sin_sb = pc.tile([HD, S], F16, name="sin_sb")
        rot_sb = pc.tile([HD, HD], F16, name="rot_sb")
        tri_sb = pc.tile([128, 128], F16, name="tri_sb")
        id_sb = pc.tile([128, 128], F16, name="id_sb")
        ones_sb = pc.tile([128, 128], F16, name="ones_sb")
        eps_sb = pc.tile([128, 1], F32, name="eps_sb")
        nc.sync.dma_start(cos_sb[:], cosT[:])
        nc.sync.dma_start(sin_sb[:], sinT[:])
        nc.sync.dma_start(rot_sb[:], rotP[:])
        nc.sync.dma_start(tri_sb[:], triM[:])
        nc.sync.dma_start(id_sb[:], idnt[:])
        nc.gpsimd.memset(ones_sb[:], 1.0)
        nc.gpsimd.memset(eps_sb[:], EPS)

        # DRAM bounce buffers for the chunked AllReduces
        pdram = ctx.enter_context(tc.tile_pool(name="pdram", bufs=1,
                                               space="DRAM"))
        ar_in = [[pdram.tile([E, SC], F16, name=f"ar_in{ph}_{c}",
                             tag=f"ari{ph}{c}")
                  for c in range(NCH)] for ph in range(2)]
        # Shared DRAM outputs are single-writer: one tile per collective
        ar_out = {}
        for l in range(L - 1):
            for ph in range(2):
                for c in range(NCH):
                    ar_out[(l, ph, c)] = pdram.tile(
                        [E, SC], F16, name=f"ar_out{l}_{ph}_{c}",
                        addr_space="Shared", tag=f"aro{l}{ph}{c}")

        # ---- rotating work pools (SBUF) ----------------------------------
        pw = ctx.enter_context(tc.tile_pool(name="pw", bufs=2))
        pn = ctx.enter_context(tc.tile_pool(name="pn", bufs=2))
        pqk = ctx.enter_context(tc.tile_pool(name="pqk", bufs=1))
        pv = ctx.enter_context(tc.tile_pool(name="pv", bufs=17))
        pat = ctx.enter_context(tc.tile_pool(name="pat", bufs=3))
        pff = ctx.enter_context(tc.tile_pool(name="pff", bufs=2))
        pio = ctx.enter_context(tc.tile_pool(name="pio", bufs=2))

        # ---- PSUM pools (long-lived; ring-buffered by tag) ---------------
        # slots are bank-padded: 4 shared [128,512] f32 accumulators + 4
        # o-accumulator banks (doubling as transpose scratch) = 8 banks
        ppb = ctx.enter_context(tc.tile_pool(name="ppb", bufs=4,
                                             space="PSUM"))
        ppo = ctx.enter_context(tc.tile_pool(name="ppo", bufs=1,
                                             space="PSUM"))

        def mm512(nm):
            return ppb.tile([128, SC], F32, name=nm, tag="mm512", bufs=4)

        def tp128(j, nm):
            return ppo.tile([128, 128], F16, name=nm, tag=f"ops{j}",
                            bufs=1)

        def rms_chunk(l, c, tag, want_rt):
            """R [128, SC] (rows all equal rsqrt(mean(x^2)+eps)) for
            position chunk c; optionally rT [128, 4] (per-partition r
            for each 128-block of the chunk)."""
            cs = slice(c * SC, (c + 1) * SC)
            ss = mm512(f"ss_{l}_{tag}_{c}")
            for e in range(ET):
                x2 = pn.tile([128, SC], F16, name=f"x2_{e}", tag="x2",
                             bufs=2)
                nc.vector.tensor_mul(x2[:], x_t[e][:, cs], x_t[e][:, cs])
                nc.tensor.matmul(ss[:], ones_sb[:], x2[:],
                                 start=(e == 0), stop=(e == ET - 1))
            R = pn.tile([128, SC], F16, name=f"R_{l}_{tag}_{c}",
                        tag=f"R{tag}", bufs=2)
            nc.scalar.activation(R[:], ss[:], AF.Abs_reciprocal_sqrt,
                                 bias=eps_sb[:], scale=1.0 / E)
            rT = None
            if want_rt:
                # per-partition r for each 128-block, via PE transpose
                rT = pn.tile([128, 4], F32, name=f"rT_{l}_{c}",
                             tag="rT", bufs=2)
                for j in range(4):
                    tp = tp128(j, f"tpr{j}")
                    nc.tensor.transpose(
                        tp[:], R[:, j * 128:(j + 1) * 128], id_sb[:])
                    nc.vector.tensor_copy(rT[:, j:j + 1], tp[:, 0:1])
            return R, rT

        def load_w_cols(dram_ap, cols, name, tag, bufs):
            """DRAM [E, cols] -> SBUF [128, ET*cols], E-tile major."""
            t = pw.tile([128, ET * cols], F16, name=name, tag=tag, bufs=bufs)
            nc.sync.dma_start(
                t[:].rearrange("p (t m) -> p t m", t=ET),
                dram_ap.rearrange("(t p) m -> p t m", p=128))
            return t

        rg = [list(range(NC))]

        # persistent v tiles [s, d | ones]: ones cols written once, the
        # value region is overwritten every layer (WAR tracked by tile)
        vext = [[pv.tile([128, 132], F16, name=f"vx_{h}_{sb}",
                         tag=f"vx{h}", bufs=ST)
                 for sb in range(ST)] for h in range(HPC)]
        for h in range(HPC):
            for sb in range(ST):
                nc.gpsimd.memset(vext[h][sb][:, 128:132], 1.0)

        for l in range(L):
            act = l < L - 1

            # per-layer weights (ring of 4 slots: wk, wq, wv, wo)
            wk_sb = load_w_cols(wk[l], DPC, f"wk_sb_{l}", "wsm", 4)
            wq_sb = load_w_cols(wq[l], DPC, f"wq_sb_{l}", "wsm", 4) if act \
                else None
            wv_sb = load_w_cols(wv[l], DPC, f"wv_sb_{l}", "wsm", 4)
            wo_sb = None
            if act:
                wo_sb = pw.tile([128, HPC * E], F16, name=f"wo_sb_{l}",
                                tag="wsm", bufs=4)
                nc.sync.dma_start(
                    wo_sb[:].rearrange("p (t m) -> p t m", t=HPC),
                    wo[l].rearrange("(t p) m -> p t m", p=128))

            # per-layer k (rope'd) tiles, whole-S, written chunk by chunk
            kr_sb = [pqk.tile([128, S], F16, name=f"kr_{l}_{h}",
                              tag=f"kr{h}", bufs=1) for h in range(HPC)] \
                if act else [None] * HPC

            # ---- attention half: per chunk norm/qkv/attn/Wo + AR fire ----
            for c in range(NCH):
                cs = slice(c * SC, (c + 1) * SC)

                # residual add from previous layer's FFN AllReduce
                if l > 0:
                    for g in range(4):
                        ld4 = pio.tile([128, 4, SC], F16, name=f"arf_{g}",
                                       tag="arl", bufs=2)
                        nc.sync.dma_start(
                            ld4[:],
                            ar_out[(l - 1, 1, c)][g * 512:(g + 1) * 512, :]
                            .rearrange("(t p) m -> p t m", p=128))
                        for i in range(4):
                            e = 4 * g + i
                            nc.vector.tensor_add(x_t[e][:, cs],
                                                 x_t[e][:, cs],
                                                 ld4[:, i, :])

                R1, rT1 = rms_chunk(l, c, "a", want_rt=True)

                # q/k projections + RoPE + k output for this chunk
                srcs = [("k", wk_sb)] + ([("q", wq_sb)] if act else [])
                for nmw, wsb in srcs:
                    for h in range(HPC):
                        if act:
                            tgt = kr_sb[h] if nmw == "k" else None
                            if nmw == "q":
                                tgt = pqk.tile([128, SC], F16,
                                               name=f"qr_{l}_{h}_{c}",
                                               tag=f"qr{h}", bufs=2)
                        ps = mm512(f"qk_{nmw}_{h}_{c}")
                        for e in range(ET):
                            nc.tensor.matmul(
                                ps[:],
                                wsb[:, e * DPC + h * 128:
                                    e * DPC + (h + 1) * 128],
                                x_t[e][:, cs],
                                start=(e == 0), stop=(e == ET - 1))
                        raw = pn.tile([128, SC], F16, name=f"raw_{h}",
                                      tag="qkraw", bufs=3)
                        nc.vector.tensor_mul(raw[:], ps[:], R1[:])
                        if nmw == "k":
                            # k output (pre-RoPE): [d, s] -> [s, d]
                            ko4 = pio.tile([128, 4, 128], F16,
                                           name=f"kos_{h}", tag="kosb",
                                           bufs=2)
                            for j in range(4):
                                tp = tp128(j, f"ko{j}")
                                nc.tensor.transpose(
                                    tp[:], raw[:, j * 128:(j + 1) * 128],
                                    id_sb[:])
                                nc.vector.tensor_copy(ko4[:, j, :], tp[:])
                            nc.sync.dma_start(
                                kvo[0, l, h, c * SC:(c + 1) * SC, :]
                                .rearrange("(t p) m -> p t m", p=128),
                                ko4[:])
                        if act:
                            # RoPE: t = raw*cos + (rotP.T @ raw)*sin
                            dst = kr_sb[h][:, cs] if nmw == "k" else tgt[:]
                            rp = mm512(f"rot_{nmw}_{h}_{c}")
                            nc.tensor.matmul(rp[:], rot_sb[:], raw[:],
                                             start=True, stop=True)
                            nc.vector.tensor_mul(dst, raw[:], cos_sb[:, cs])
                            tmp = pn.tile([128, SC], F16, name=f"rtmp_{h}",
                                          tag="rtmp", bufs=2)
                            nc.vector.tensor_mul(tmp[:], rp[:],
                                                 sin_sb[:, cs])
                            nc.vector.tensor_add(dst, dst, tmp[:])
                            if nmw == "q":
                                qr_c = tgt
                                if h == 0:
                                    qr_sb = [None] * HPC
                                qr_sb[h] = qr_c

                # v for this chunk's 4 blocks, [s, d] + ones col + output
                for j in range(4):
                    sb = c * 4 + j
                    ps = mm512(f"v_{sb}")
                    for e in range(ET):
                        nc.tensor.matmul(
                            ps[:, 0:DPC], x_t[e][:, sb * 128:(sb + 1) * 128],
                            wv_sb[:, e * DPC:(e + 1) * DPC],
                            start=(e == 0), stop=(e == ET - 1))
                    for h in range(HPC):
                        vt = vext[h][sb]
                        nc.vector.tensor_scalar_mul(
                            vt[:, 0:128], ps[:, h * 128:(h + 1) * 128],
                            rT1[:, j:j + 1])
                        nc.sync.dma_start(
                            kvo[1, l, h, sb * 128:(sb + 1) * 128, :],
                            vt[:, 0:128])

                if not act:
                    continue

                # ---- attention for q-chunk c (scores transposed) --------
                oT = []
                for h in range(HPC):
                    ot = pqk.tile([128, SC], F16, name=f"oT_{l}_{h}_{c}",
                                  tag=f"oT{h}", bufs=2)
                    ops = [ppo.tile([128, 132], F32, name=f"ops{j}",
                                    tag=f"ops{j}", bufs=1)
                           for j in range(4)]
                    nkb = 4 * c + 4
                    for kb in range(nkb):
                        st = mm512(f"st_{h}_{kb}")
                        nc.tensor.matmul(
                            st[:], kr_sb[h][:, kb * 128:(kb + 1) * 128],
                            qr_sb[h][:], start=True, stop=True)
                        ex = pat.tile([128, SC], F16, name=f"ex_{kb}",
                                      tag="ex")
                        nc.scalar.activation(ex[:], st[:], AF.Exp,
                                             scale=SCALE)
                        for j in range(4):
                            qb = 4 * c + j
                            if qb < kb:
                                continue
                            exs = ex[:, j * 128:(j + 1) * 128]
                            if qb == kb:
                                nc.vector.tensor_mul(exs, exs, tri_sb[:])
                            nc.tensor.matmul(
                                ops[j], exs, vext[h][kb][:],
                                start=(kb == 0), stop=(kb == nkb - 1))
                    for j in range(4):
                        rec = pn.tile([128, 1], F32, name=f"rec{j}",
                                      tag="rec", bufs=2)
                        nc.vector.reciprocal(rec[:], ops[j][:, 128:129])
                        ob = pio.tile([128, 128], F16, name=f"ob{j}",
                                      tag="ob", bufs=2)
                        nc.vector.tensor_scalar_mul(ob[:], ops[j][:, 0:128],
                                                    rec[:])
                        tp = tp128(j, f"to{j}")
                        nc.tensor.transpose(tp[:], ob[:], id_sb[:])
                        nc.vector.tensor_copy(
                            ot[:, j * 128:(j + 1) * 128], tp[:])
                    oT.append(ot)

                # ---- Wo partial for chunk c + fire AllReduce ------------
                # casts alternate DVE/ACT per 4-m group; one store each
                for g in range(4):
                    cst4 = pio.tile([128, 4, SC], F16, name=f"woc_{g}",
                                    tag="cast", bufs=2)
                    on_act = (g % 2 == 0)
                    for i in range(4):
                        m = 4 * g + i
                        ps = mm512(f"wo_{m}")
                        for h in range(HPC):
                            nc.tensor.matmul(
                                ps[:],
                                wo_sb[:, h * E + m * 128:
                                      h * E + (m + 1) * 128],
                                oT[h][:], start=(h == 0),
                                stop=(h == HPC - 1))
                        if on_act:
                            nc.scalar.copy(cst4[:, i, :], ps[:])
                        else:
                            nc.vector.tensor_copy(cst4[:, i, :], ps[:])
                    eng = nc.scalar if on_act else nc.sync
                    eng.dma_start(
                        ar_in[0][c][g * 512:(g + 1) * 512, :]
                        .rearrange("(t p) m -> p t m", p=128), cst4[:])
                nc.gpsimd.collective_compute(
                    "AllReduce", mybir.AluOpType.add, replica_groups=rg,
                    ins=[ar_in[0][c].opt()], outs=[ar_out[(l, 0, c)].opt()])

            if not act:
                continue

            # ---- FFN half: chunk-PAIRS share one weight load ------------
            for half in range(NCH // 2):
                cpair = (2 * half, 2 * half + 1)
                R2s = {}
                for c in cpair:
                    cs = slice(c * SC, (c + 1) * SC)
                    for g in range(4):
                        ld4 = pio.tile([128, 4, SC], F16, name=f"ara_{g}",
                                       tag="arl", bufs=2)
                        nc.sync.dma_start(
                            ld4[:],
                            ar_out[(l, 0, c)][g * 512:(g + 1) * 512, :]
                            .rearrange("(t p) m -> p t m", p=128))
                        for i in range(4):
                            e = 4 * g + i
                            nc.vector.tensor_add(x_t[e][:, cs],
                                                 x_t[e][:, cs],
                                                 ld4[:, i, :])
                    R2s[c], _ = rms_chunk(l, c, "f", want_rt=False)

                m_sb = {c: [] for c in cpair}
                for fm in range(FT + 1):
                    rows = 128 if fm < FT else FPC - FT * 128
                    wgs = pff.tile([128, ET * rows], F16, name=f"wgs{fm}",
                                   tag="wgs", bufs=2)
                    nc.sync.dma_start(
                        wgs[:].rearrange("p (t m) -> p t m", t=ET),
                        wg[l][:, fm * 128:fm * 128 + rows].rearrange(
                            "(t p) m -> p t m", p=128))
                    wus = pff.tile([128, ET * rows], F16, name=f"wus{fm}",
                                   tag="wus", bufs=2)
                    nc.sync.dma_start(
                        wus[:].rearrange("p (t m) -> p t m", t=ET),
                        wu[l][:, fm * 128:fm * 128 + rows].rearrange(
                            "(t p) m -> p t m", p=128))
                    pp = {}
                    for c in cpair:
                        cs = slice(c * SC, (c + 1) * SC)
                        gp = mm512(f"g_{fm}_{c}")
                        up = mm512(f"u_{fm}_{c}")
                        for e in range(ET):
                            nc.tensor.matmul(
                                gp[0:rows, :],
                                wgs[:, e * rows:(e + 1) * rows],
                                x_t[e][:, cs], start=(e == 0),
                                stop=(e == ET - 1))
                        for e in range(ET):
                            nc.tensor.matmul(
                                up[0:rows, :],
                                wus[:, e * rows:(e + 1) * rows],
                                x_t[e][:, cs], start=(e == 0),
                                stop=(e == ET - 1))
                        pp[c] = (gp, up)
                    for c in cpair:
                        gp, up = pp[c]
                        gs = pff.tile([128, SC], F16, name=f"gs{fm}",
                                      tag="gs", bufs=2)
                        us = pff.tile([128, SC], F16, name=f"us{fm}",
                                      tag="us", bufs=2)
                        mt = pff.tile([128, SC], F16, name=f"m_{fm}_{c}",
                                      tag="mff", bufs=13)
                        nc.vector.tensor_mul(gs[0:rows, :], gp[0:rows, :],
                                             R2s[c][0:rows, :])
                        nc.scalar.activation(gs[0:rows, :], gs[0:rows, :],
                                             AF.Silu)
                        nc.vector.tensor_mul(us[0:rows, :], up[0:rows, :],
                                             R2s[c][0:rows, :])
                        nc.vector.tensor_mul(mt[0:rows, :], gs[0:rows, :],
                                             us[0:rows, :])
                        m_sb[c].append(mt)

                # down-proj partials (wd loaded 4 m-cols at a time,
                # reused by both chunks of the pair)
                for g in range(4):
                    # [p, fm-tile, 4 m-blocks x 128] — 3D-balanceable DMA
                    wds4 = pff.tile([128, FT + 1, 512], F16,
                                    name=f"wds{g}", tag="wds", bufs=2)
                    nc.sync.dma_start(
                        wds4[:, 0:FT, :],
                        wd[l][0:FT * 128, g * 512:(g + 1) * 512]
                        .rearrange("(t p) m -> p t m", p=128))
                    nc.sync.dma_start(
                        wds4[0:FPC - FT * 128, FT, :],
                        wd[l][FT * 128:FPC, g * 512:(g + 1) * 512])
                    for c in cpair:
                        cst4 = pio.tile([128, 4, SC], F16, name=f"dnc_{g}",
                                        tag="cast", bufs=2)
                        on_act = ((g + c) % 2 == 0)
                        for i in range(4):
                            m = 4 * g + i
                            ps = mm512(f"dn_{m}")
                            for fm in range(FT + 1):
                                rows = 128 if fm < FT else FPC - FT * 128
                                nc.tensor.matmul(
                                    ps[:],
                                    wds4[0:rows, fm,
                                         i * 128:(i + 1) * 128],
                                    m_sb[c][fm][0:rows, :],
                                    start=(fm == 0), stop=(fm == FT))
                            if on_act:
                                nc.scalar.copy(cst4[:, i, :], ps[:])
                            else:
                                nc.vector.tensor_copy(cst4[:, i, :], ps[:])
                        eng = nc.scalar if on_act else nc.sync
                        eng.dma_start(
                            ar_in[1][c][g * 512:(g + 1) * 512, :]
                            .rearrange("(t p) m -> p t m", p=128), cst4[:])
                for c in cpair:
                    nc.gpsimd.collective_compute(
                        "AllReduce", mybir.AluOpType.add, replica_groups=rg,
                        ins=[ar_in[1][c].opt()],
                        outs=[ar_out[(l, 1, c)].opt()])

    nc.compile()
    return nc


def _host_prep(inputs):
    """Fold norms into weights, build tables, TP-shard -> per-core in_maps."""
    ids = np.asarray(inputs["input_ids"]).reshape(-1)
    x0 = np.asarray(inputs["embed"])[ids]          # [S, E] fp32
    x0T = np.ascontiguousarray(x0.T).astype(np.float16)

    ln1 = np.asarray(inputs["ln1"], dtype=np.float32)   # [L, E]
    ln2 = np.asarray(inputs["ln2"], dtype=np.float32)
    wq_f = ln1[:, :, None] * np.asarray(inputs["Wq"])   # [L, E, H*HD]
    wk_f = ln1[:, :, None] * np.asarray(inputs["Wk"])
    wv_f = ln1[:, :, None] * np.asarray(inputs["Wv"])
    wg_f = ln2[:, :, None] * np.asarray(inputs["Wg"])
    wu_f = ln2[:, :, None] * np.asarray(inputs["Wu"])
    wo_f = np.asarray(inputs["Wo"])                     # [L, H*HD, E]
    wd_f = np.asarray(inputs["Wd"])                     # [L, FF, E]

    inv = 1.0 / (ROPE_THETA ** (np.arange(0, HD, 2, dtype=np.float32) / HD))
    t = np.arange(S, dtype=np.float32)
    freqs = np.outer(t, inv)                       # [S, HD/2]
    emb = np.concatenate([freqs, freqs], axis=-1)  # [S, HD]
    cosT = np.ascontiguousarray(np.cos(emb).T).astype(np.float16)
    sinT = np.ascontiguousarray(np.sin(emb).T).astype(np.float16)

    rotP = np.zeros((HD, HD), dtype=np.float16)
    half = HD // 2
    for d in range(half):
        rotP[d + half, d] = -1.0
    for d in range(half, HD):
        rotP[d - half, d] = 1.0

    triM = np.triu(np.ones((128, 128), dtype=np.float16))   # [k, q] valid
    idnt = np.eye(128, dtype=np.float16)

    in_maps = []
    for c in range(NC):
        ds = slice(c * DPC, (c + 1) * DPC)
        fs = slice(c * FPC, (c + 1) * FPC)
        in_maps.append({
            "x0T": x0T,
            "wq": np.ascontiguousarray(wq_f[:, :, ds]).astype(np.float16),
            "wk": np.ascontiguousarray(wk_f[:, :, ds]).astype(np.float16),
            "wv": np.ascontiguousarray(wv_f[:, :, ds]).astype(np.float16),
            "wo": np.ascontiguousarray(wo_f[:, ds, :]).astype(np.float16),
            "wg": np.ascontiguousarray(wg_f[:, :, fs]).astype(np.float16),
            "wu": np.ascontiguousarray(wu_f[:, :, fs]).astype(np.float16),
            "wd": np.ascontiguousarray(wd_f[:, fs, :]).astype(np.float16),
            "cosT": cosT, "sinT": sinT, "rotP": rotP,
            "triM": triM, "idnt": idnt,
        })
    return in_maps


def kernel(**inputs):
    if "nc" not in _CACHE:
        _CACHE["nc"] = build_kernel()
    nc = _CACHE["nc"]
    in_maps = _host_prep(inputs)
    trace = os.environ.get("KERNEL_TRACE") == "1"
    res = run_bass_kernel_spmd(nc, in_maps, core_ids=list(range(NC)),
                               trace=trace)
    if trace and res.exec_time_ns is not None:
        print(f"HW exec time: {res.exec_time_ns} ns")
        _CACHE["exec_time_ns"] = res.exec_time_ns
        if res.instructions_and_trace:
            print("trace:", res.instructions_and_trace[1])

    out = np.zeros((2, L, B, H, S, HD), dtype=np.float32)
    for c in range(NC):
        kv = res.results[c]["kv_out"].astype(np.float32)  # [2, L, HPC, S, HD]
        for h in range(HPC):
            out[:, :, 0, c * HPC + h] = kv[:, :, h]
    return out
